# revision 1
# baseline (speedup 1.0000x reference)
"""Trainium2 Bass kernel for nn_CSACMRFusion (deformable-conv + CARAFE fusion).

Self-contained: accepts FULL unsharded inputs, shards batch across 8 cores
(1 sample/core), runs one Bass/Tile kernel per core via run_bass_kernel_spmd,
gathers the full output.

Per-core pipeline (all on-chip math fp16, PSUM accumulate f32):
  cf conv (classic matmul)  -> X_off (ck-part, padded)
  Y_k = al_w[:,:,k] @ X_in  -> swapped matmuls -> Y^T px-major in DRAM
  e1, off, e2 convs (classic) -> PE-transpose -> offsets/kern in px-part
  softmax (px-part, batched), deform index/weight math (px-part, batched)
  dma_gather of Y^T rows (4 bilinear corners = AP offsets {0,1,64,65})
  fused scalar_tensor_tensor MACs -> X_align^T -> DRAM
  CARAFE: shifted DMA reads of X_align^T + kern-weighted STT MACs -> X_down^T
  xbar DMA transpose back to ck-part; fo conv (classic) + LeakyReLU -> out
"""

import numpy as np

P = 128
H = W = 64
HW = 4096
NG = 32          # px groups: px = g*128 + p
CI = 256
CO = 256
CMID = 64
K9 = 9
PADW = 66
PADHW = PADW * PADW  # 4356
NCHUNK = 8       # spatial chunks of 8 rows = 512 px for classic convs
GCHUNK = 4       # gather chunks of 1024 px
MAGIC = 12582912.0  # 3 * 2**22, f32 round-to-int magic

_CACHE = {}


def _taps():
    return [(k // 3 - 1, k % 3 - 1) for k in range(9)]


def build_kernel(debug=False):
    import concourse.bass as bass
    import concourse.tile as tile
    from concourse import bacc, mybir
    from concourse.masks import make_identity

    f16 = mybir.dt.float16
    f32 = mybir.dt.float32
    i16 = mybir.dt.int16
    ALU = mybir.AluOpType
    ACT = mybir.ActivationFunctionType

    nc = bacc.Bacc("TRN2", target_bir_lowering=False, debug=False, num_devices=8)

    # ---------------- DRAM I/O ----------------
    xo_d = nc.dram_tensor("xo", [2, P, HW], f16, kind="ExternalInput")
    xi_d = nc.dram_tensor("xi", [2, P, HW], f16, kind="ExternalInput")
    wcf_d = nc.dram_tensor("w_cf", [P, 9 * 4 * 2 * P], f16, kind="ExternalInput")
    wal_d = nc.dram_tensor("w_al", [P, 2 * 9 * CO], f16, kind="ExternalInput")
    woff_d = nc.dram_tensor("w_off", [P, 9 * 2 * 18], f16, kind="ExternalInput")
    we1_d = nc.dram_tensor("w_e1", [P, 9 * 2 * CMID], f16, kind="ExternalInput")
    we2_d = nc.dram_tensor("w_e2", [CMID, 9 * 9], f16, kind="ExternalInput")
    wfo_d = nc.dram_tensor("w_fo", [P, 9 * 6 * 2 * P], f16, kind="ExternalInput")
    bcf_d = nc.dram_tensor("b_cf", [P, 2], f32, kind="ExternalInput")
    boff_d = nc.dram_tensor("b_off", [18, 1], f32, kind="ExternalInput")
    be1_d = nc.dram_tensor("b_e1", [CMID, 1], f32, kind="ExternalInput")
    be2_d = nc.dram_tensor("b_e2", [9, 1], f32, kind="ExternalInput")
    bfo_d = nc.dram_tensor("b_fo", [P, 2], f32, kind="ExternalInput")
    bcfn_d = nc.dram_tensor("b_cf_n", [P, 2], f32, kind="ExternalInput")
    bfon_d = nc.dram_tensor("b_fo_n", [P, 2], f32, kind="ExternalInput")
    alb_d = nc.dram_tensor("alb_rep", [P, CO], f16, kind="ExternalInput")
    vmask_d = nc.dram_tensor("vmask", [P, NG * 9], f16, kind="ExternalInput")
    gkiy_d = nc.dram_tensor("gkiy", [P, 9 * NG], f16, kind="ExternalInput")
    gkjx_d = nc.dram_tensor("gkjx", [P, 9 * NG], f16, kind="ExternalInput")
    out_d = nc.dram_tensor("out", [2, P, HW], f32, kind="ExternalOutput")
    dbg = {}
    if debug:
        dbg["okT"] = nc.dram_tensor("d_okT", [P, NG * 27], f32, kind="ExternalOutput")
        dbg["kern"] = nc.dram_tensor("d_kern", [P, NG * 9], f32, kind="ExternalOutput")
        dbg["xal"] = nc.dram_tensor("d_xal", [P, NG * CO], f16, kind="ExternalOutput")
        dbg["xd"] = nc.dram_tensor("d_xd", [P, NG * CO], f16, kind="ExternalOutput")
        dbg["xoff"] = nc.dram_tensor("d_xoff", [2, P, HW], f16, kind="ExternalOutput")
        dbg["idx"] = nc.dram_tensor("d_idx", [P, 9 * NG], f32, kind="ExternalOutput")
        dbg["wc0"] = nc.dram_tensor("d_wc0", [P, 9 * NG], f32, kind="ExternalOutput")
        dbg["wt"] = nc.dram_tensor("d_wt", [P, 9 * 256], np.int16 if False else mybir.dt.int16, kind="ExternalOutput")

    taps = _taps()

    with tile.TileContext(nc) as tc:
        with (
            tc.tile_pool(name="persist", bufs=1) as pp,
            tc.tile_pool(name="ystage", bufs=2) as ysp,
            tc.tile_pool(name="stage32", bufs=2) as s32p,
            tc.tile_pool(name="gdst", bufs=2) as gdp,
            tc.tile_pool(name="cdst", bufs=2) as cdp,
            tc.tile_pool(name="pmain", bufs=2, space="PSUM") as pmm,
            tc.tile_pool(name="py", bufs=4, space="PSUM") as pyy,
            tc.tile_pool(name="psmall", bufs=2, space="PSUM") as psm,
            tc.tile_pool(name="dram", bufs=1, space="DRAM") as dp,
        ):
            # ---------------- persistent SBUF ----------------
            xcat = pp.tile([P, 4, PADHW], f16, tag="xcat")
            xoff = pp.tile([P, 2, PADHW], f16, tag="xoff")
            e1p = pp.tile([CMID, PADHW], f16, tag="e1p")
            wcf = pp.tile([P, 9 * 4 * 2 * P], f16, tag="wcf")
            wal = pp.tile([P, 2 * 9 * CO], f16, tag="wal")
            woff = pp.tile([P, 9 * 2 * 18], f16, tag="woff")
            we1 = pp.tile([P, 9 * 2 * CMID], f16, tag="we1")
            we2 = pp.tile([CMID, 9 * 9], f16, tag="we2")
            wfo = pp.tile([P, 9 * 6 * 2 * P], f16, tag="wfo")
            bcf = pp.tile([P, 2], f32, tag="bcf")
            boff = pp.tile([18, 1], f32, tag="boff")
            be1 = pp.tile([CMID, 1], f32, tag="be1")
            be2 = pp.tile([9, 1], f32, tag="be2")
            bfo = pp.tile([P, 2], f32, tag="bfo")
            bcfn = pp.tile([P, 2], f32, tag="bcfn")
            bfon = pp.tile([P, 2], f32, tag="bfon")
            albr = pp.tile([P, CO], f16, tag="albr")
            vmask = pp.tile([P, NG, 9], f16, tag="vmask")
            gkiy = pp.tile([P, 9, NG], f16, tag="gkiy")
            gkjx = pp.tile([P, 9, NG], f16, tag="gkjx")
            ident = pp.tile([P, P], f32, tag="ident")
            okT = pp.tile([P, NG, 27], f32, tag="okT")
            zeros = pp.tile([P, 2304], f16, tag="zeros")
            # deform math tensors [128, 9, 32] f32
            ys = pp.tile([P, 9, NG], f32, tag="ys")
            xs = pp.tile([P, 9, NG], f32, tag="xs")
            y0f = pp.tile([P, 9, NG], f32, tag="y0f")
            x0f = pp.tile([P, 9, NG], f32, tag="x0f")
            fy = pp.tile([P, 9, NG], f32, tag="fy")
            fx = pp.tile([P, 9, NG], f32, tag="fx")
            tmpa = pp.tile([P, 9, NG], f32, tag="tmpa")
            tmpb = pp.tile([P, 9, NG], f32, tag="tmpb")
            wy0 = pp.tile([P, 9, NG], f32, tag="wy0")
            wy1 = pp.tile([P, 9, NG], f32, tag="wy1")
            wx0 = pp.tile([P, 9, NG], f32, tag="wx0")
            wx1 = pp.tile([P, 9, NG], f32, tag="wx1")
            wc = [pp.tile([P, 9, NG], f32, tag=f"wc{i}", name=f"wc{i}") for i in range(4)]
            idxf = pp.tile([P, 9, NG], f32, tag="idxf")
            tsb = [pp.tile([P, P], f32, tag=f"tsb{j}", name=f"tsb{j}") for j in range(3)]
            WT = pp.tile([P, 9, 256], i16, tag="e1p", name="WT")
            expt = pp.tile([P, NG, 9], f32, tag="expt")
            den = pp.tile([P, NG, 1], f32, tag="den")
            rec = pp.tile([P, NG, 1], f32, tag="rec")
            keff = pp.tile([P, NG, 9], f32, tag="keff")
            xal = pp.tile([P, NG, CO], f16, tag="wal", name="xal")
            xdn = pp.tile([P, NG, CO], f16, tag="xdn")

            # ---------------- DRAM scratch ----------------
            yt = dp.tile([4226, 9 * CO], f16, tag="yt")
            xalt = dp.tile([4226, CO], f16, tag="xalt")
            xdt = dp.tile([HW, CO], f16, tag="xdt")

            def interior(padtile, ci):
                return padtile[:, ci, :].rearrange("p (h w) -> p h w", h=PADW)

            def rhs_ap(padtile, ci, r0, dy, dx, nr=8):
                # [Ppart, nr rows, 64] shifted view inside padded image
                v = interior(padtile, ci)
                return v[:, 1 + r0 + dy : 1 + r0 + nr + dy, 1 + dx : 65 + dx]

            def e1_interior():
                return e1p[:, :].rearrange("p (h w) -> p h w", h=PADW)

            STT0 = nc.vector.scalar_tensor_tensor

            # ---------------- P0: loads + memsets ----------------
            make_identity(nc, ident[:])
            nc.vector.memset(zeros[:], 0)
            for t_ in (xcat, xoff):
                nc.gpsimd.memset(t_[:], 0)
            nc.gpsimd.memset(e1p[:], 0)
            nc.sync.dma_start(wcf[:], wcf_d[:])
            nc.sync.dma_start(wal[:], wal_d[:])
            nc.sync.dma_start(woff[:], woff_d[:])
            nc.sync.dma_start(we1[:], we1_d[:])
            nc.sync.dma_start(we2[:], we2_d[:])
            nc.sync.dma_start(wfo[:], wfo_d[:])
            for sb, dr in ((bcf, bcf_d), (boff, boff_d), (be1, be1_d),
                           (be2, be2_d), (bfo, bfo_d), (albr, alb_d),
                           (bcfn, bcfn_d), (bfon, bfon_d)):
                nc.sync.dma_start(sb[:], dr[:])
            nc.sync.dma_start(vmask[:].rearrange("p g k -> p (g k)"), vmask_d[:])
            nc.sync.dma_start(gkiy[:].rearrange("p k g -> p (k g)"), gkiy_d[:])
            nc.sync.dma_start(gkjx[:].rearrange("p k g -> p (k g)"), gkjx_d[:])
            # inputs into padded interiors
            for ci in range(2):
                nc.sync.dma_start(
                    interior(xcat, ci)[:, 1:65, 1:65],
                    xo_d[ci, :, :].rearrange("p (h w) -> p h w", h=64),
                )
                nc.sync.dma_start(
                    interior(xcat, 2 + ci)[:, 1:65, 1:65],
                    xi_d[ci, :, :].rearrange("p (h w) -> p h w", h=64),
                )
            # zero margins of DRAM scratch
            nc.sync.dma_start(yt[0:65, :], zeros[0:65, :])
            nc.sync.dma_start(yt[4161:4226, :], zeros[0:65, :])
            nc.sync.dma_start(xalt[0:65, :], zeros[0:65, 0:CO])
            nc.sync.dma_start(xalt[4161:4226, :], zeros[0:65, 0:CO])

            # ---------------- P1: cf conv ----------------
            for cot in range(2):
                for c in range(NCHUNK):
                    ps = pmm.tile([P, 512], f32, tag="pmm")
                    n = 0
                    for t in range(9):
                        dy, dx = taps[t]
                        for cit in range(4):
                            j = ((t * 4 + cit) * 2 + cot) * P
                            nc.tensor.matmul(
                                ps[:],
                                lhsT=wcf[:, j : j + P],
                                rhs=rhs_ap(xcat, cit, c * 8, dy, dx),
                                start=(n == 0),
                                stop=(n == 35),
                            )
                            n += 1
                    dstv = rhs_ap(xoff, cot, c * 8, 0, 0)
                    nc.scalar.activation(
                        dstv, ps[:].rearrange("p (a b) -> p a b", a=8),
                        ACT.Relu, bias=bcf[:, cot : cot + 1],
                    )
                    rneg = s32p.tile([P, 512], f16, tag="st32",
                                     name=f"rncf_{cot}_{c}")
                    nc.scalar.activation(
                        rneg[:], ps[:], ACT.Relu,
                        bias=bcfn[:, cot : cot + 1], scale=-1.0,
                    )
                    STT0(dstv, rneg[:].rearrange("p (a b) -> p a b", a=8),
                         -0.1, dstv, ALU.mult, ALU.add)

            # ---------------- P2: Y_k swapped matmuls ----------------
            # stationary operand needs a single contiguous free dim -> stage
            ysegs = [(0, 512), (512, 512), (1024, 512), (1536, 512), (2048, 256)]
            for g in range(NG):
                yst = ysp.tile([P, 2304], f16, tag="yst", name=f"yst{g}")
                lsts = []
                for cit in range(2):
                    lst = ysp.tile([P, 128], f16, tag=f"lst{cit}", name=f"lst{cit}_{g}")
                    nc.vector.tensor_copy(
                        lst[:].rearrange("p (a b) -> p a b", a=2),
                        rhs_ap(xcat, 2 + cit, g * 2, 0, 0, nr=2),
                    )
                    lsts.append(lst)
                for si, (o0, nn) in enumerate(ysegs):
                    ps = pyy.tile([P, 512], f32, tag="pyq", name=f"pyq{g}_{si}")
                    for cit in range(2):
                        nc.tensor.matmul(
                            ps[:, 0:nn],
                            lhsT=lsts[cit][:],
                            rhs=wal[:, cit * 2304 + o0 : cit * 2304 + o0 + nn],
                            start=(cit == 0),
                            stop=(cit == 1),
                        )
                    nc.scalar.activation(
                        yst[:, o0 : o0 + nn], ps[:, 0:nn], ACT.Copy
                    )
                nc.sync.dma_start(yt[65 + g * 128 : 65 + (g + 1) * 128, :], yst[:])

            # ---------------- P3a: e1 conv ----------------
            for c in range(NCHUNK):
                ps = psm.tile([CMID, 512], f32, tag="sm", name=f"pe1_{c}")
                n = 0
                for t in range(9):
                    dy, dx = taps[t]
                    for cit in range(2):
                        j = (t * 2 + cit) * CMID
                        nc.tensor.matmul(
                            ps[:],
                            lhsT=we1[:, j : j + CMID],
                            rhs=rhs_ap(xoff, cit, c * 8, dy, dx),
                            start=(n == 0),
                            stop=(n == 17),
                        )
                        n += 1
                ei = e1_interior()
                nc.scalar.activation(
                    ei[:, 1 + c * 8 : 9 + c * 8, 1:65],
                    ps[:].rearrange("p (a b) -> p a b", a=8),
                    ACT.Identity,
                    bias=be1[:, 0:1],
                )

            # ---------------- P3b: off + e2 convs, transpose to px-part ----------------
            for c in range(NCHUNK):
                pso = psm.tile([18, 512], f32, tag="sm", name=f"poff_{c}")
                n = 0
                for t in range(9):
                    dy, dx = taps[t]
                    for cit in range(2):
                        j = (t * 2 + cit) * 18
                        nc.tensor.matmul(
                            pso[:],
                            lhsT=woff[:, j : j + 18],
                            rhs=rhs_ap(xoff, cit, c * 8, dy, dx),
                            start=(n == 0),
                            stop=(n == 17),
                        )
                        n += 1
                pse = psm.tile([9, 512], f32, tag="sm", name=f"pe2_{c}")
                ei = e1_interior()
                for t in range(9):
                    dy, dx = taps[t]
                    nc.tensor.matmul(
                        pse[:],
                        lhsT=we2[:, t * 9 : (t + 1) * 9],
                        rhs=ei[:, 1 + c * 8 + dy : 9 + c * 8 + dy,
                               1 + dx : 65 + dx],
                        start=(t == 0),
                        stop=(t == 8),
                    )
                st = s32p.tile([18, 512], f32, tag="st32", name=f"sto_{c}")
                nc.vector.tensor_scalar(
                    st[:], pso[:], boff[:, 0:1], -16.0, ALU.add, ALU.max
                )
                nc.vector.tensor_scalar(
                    st[:], st[:], 16.0, None, ALU.min
                )
                stk = s32p.tile([9, 512], f32, tag="st32", name=f"stk_{c}")
                nc.scalar.activation(
                    stk[:], pse[:], ACT.Identity, bias=be2[:, 0:1]
                )
                for q in range(4):
                    pt = psm.tile([P, 32], f32, tag="sm", name=f"ptr_{c}_{q}")
                    nc.tensor.transpose(
                        pt[:, 0:18], st[:, q * 128 : (q + 1) * 128], ident[0:18, 0:18]
                    )
                    nc.vector.tensor_copy(okT[:, c * 4 + q, 0:18], pt[:, 0:18])
                    pt2 = psm.tile([P, 32], f32, tag="sm", name=f"ptk_{c}_{q}")
                    nc.tensor.transpose(
                        pt2[:, 0:9], stk[:, q * 128 : (q + 1) * 128], ident[0:9, 0:9]
                    )
                    nc.vector.tensor_copy(okT[:, c * 4 + q, 18:27], pt2[:, 0:9])

            # ---------------- P4: softmax + deform index math ----------------
            nc.scalar.activation(expt[:], okT[:, :, 18:27], ACT.Exp)
            nc.vector.tensor_reduce(den[:], expt[:], axis=mybir.AxisListType.X,
                                    op=ALU.add)
            nc.vector.reciprocal(rec[:], den[:])
            for g in range(NG):
                nc.vector.tensor_scalar(
                    keff[:, g, :], expt[:, g, :], rec[:, g, 0:1], None, ALU.mult
                )
            nc.vector.tensor_tensor(
                out=keff[:], in0=keff[:], in1=vmask[:], op=ALU.mult
            )

            # offsets: okT ch 2k = dy_k, 2k+1 = dx_k ; view as [p, k, g]
            okv = okT[:].rearrange("p g c -> p c g")
            dys = okv[:, 0:18:2, :]
            dxs = okv[:, 1:18:2, :]
            TT = nc.vector.tensor_tensor
            TS = nc.vector.tensor_scalar
            STT = nc.vector.scalar_tensor_tensor
            fl = lambda t_: t_[:]
            TT(out=fl(ys), in0=dys, in1=fl(gkiy), op=ALU.add)
            TT(out=fl(xs), in0=dxs, in1=fl(gkjx), op=ALU.add)
            TS(fl(y0f), fl(ys), MAGIC, MAGIC, ALU.add, ALU.subtract)
            TS(fl(x0f), fl(xs), MAGIC, MAGIC, ALU.add, ALU.subtract)
            STT(fl(fy), fl(ys), 0.5, fl(y0f), ALU.add, ALU.subtract)
            STT(fl(fx), fl(xs), 0.5, fl(x0f), ALU.add, ALU.subtract)

            def valid(dst, src, lo, hi):
                TS(fl(tmpa), fl(src), lo, None, ALU.is_ge)
                TS(fl(tmpb), fl(src), hi, None, ALU.is_le)
                TT(out=fl(dst), in0=fl(tmpa), in1=fl(tmpb), op=ALU.mult)

            # wy0 = (1-fy)*valid(y0), wy1 = fy*valid(y1) ; same for x
            valid(wy0, y0f, 0.0, 63.0)
            valid(wy1, y0f, -1.0, 62.0)
            valid(wx0, x0f, 0.0, 63.0)
            valid(wx1, x0f, -1.0, 62.0)
            TS(fl(tmpa), fl(fy), -1.0, 1.0, ALU.mult, ALU.add)  # 1-fy
            TT(out=fl(wy0), in0=fl(wy0), in1=fl(tmpa), op=ALU.mult)
            TT(out=fl(wy1), in0=fl(wy1), in1=fl(fy), op=ALU.mult)
            TS(fl(tmpa), fl(fx), -1.0, 1.0, ALU.mult, ALU.add)  # 1-fx
            TT(out=fl(wx0), in0=fl(wx0), in1=fl(tmpa), op=ALU.mult)
            TT(out=fl(wx1), in0=fl(wx1), in1=fl(fx), op=ALU.mult)
            TT(out=fl(wc[0]), in0=fl(wy0), in1=fl(wx0), op=ALU.mult)
            TT(out=fl(wc[1]), in0=fl(wy0), in1=fl(wx1), op=ALU.mult)
            TT(out=fl(wc[2]), in0=fl(wy1), in1=fl(wx0), op=ALU.mult)
            TT(out=fl(wc[3]), in0=fl(wy1), in1=fl(wx1), op=ALU.mult)
            # base corner index with 65-row front margin:
            # idx = 64*clip(y0+1, 0, 64) + clip(x0+1, 0, 64)
            TS(fl(tmpa), fl(y0f), 1.0, 0.0, ALU.add, ALU.max)
            TS(fl(tmpa), fl(tmpa), 64.0, None, ALU.min)
            TS(fl(tmpb), fl(x0f), 1.0, 0.0, ALU.add, ALU.max)
            TS(fl(tmpb), fl(tmpb), 64.0, None, ALU.min)
            STT(fl(idxf), fl(tmpa), 64.0, fl(tmpb), ALU.mult, ALU.add)

            # ---------------- P4b: wrap fold idx -> [16-rep, 9, 256] int16 ----------------
            chunks = [(0, 4), (4, 4), (8, 1)]
            for j, (k0, nk) in enumerate(chunks):
                pt1 = psm.tile([P, P], f32, tag="sm", name=f"pt1_{j}")
                nc.tensor.transpose(
                    pt1[0 : nk * 32, 0:P],
                    idxf[:, k0 : k0 + nk, :].rearrange("p a b -> p (a b)"),
                    ident[:],
                )
                nc.vector.tensor_copy(tsb[j][0 : nk * 32, :], pt1[0 : nk * 32, :])
            for j, (k0, nk) in enumerate(chunks):
                for u in range(8):
                    pt2 = psm.tile([16, P], f32, tag="sm", name=f"pt2_{j}_{u}")
                    nc.tensor.transpose(
                        pt2[:, 0 : nk * 32],
                        tsb[j][0 : nk * 32, 16 * u : 16 * u + 16],
                        ident[0 : nk * 32, 0 : nk * 32],
                    )
                    # strided dest: [16 part, nk (step 256), 32 (step 8)] offset u+k0*256
                    nc.vector.tensor_copy(
                        WT[0:16, k0 : k0 + nk, u : u + 249 : 8], pt2[:, 0 : nk * 32]
                    )
            for r in range(1, 8):
                nc.sync.dma_start(
                    WT[16 * r : 16 * r + 16, :, :], WT[0:16, :, :]
                )

            # ---------------- P5: deform gathers + STT accumulate ----------------
            deltas = (0, 1, 64, 65)
            GCH, GPC = 8, 4  # 8 chunks of 512 px
            for c in range(GCH):
                for k in range(9):
                    gts = []
                    for cor in range(4):
                        gt = gdp.tile([P, GPC, CO], f16, tag=f"gd{cor}", name=f"gd{cor}_{c}_{k}")
                        d = deltas[cor]
                        nc.gpsimd.dma_gather(
                            gt[:],
                            yt[d : d + 4161, k * CO : (k + 1) * CO],
                            WT[:, k, c * 32 : (c + 1) * 32],
                            512,
                            512,
                            CO,
                            elem_step=9 * CO,
                        )
                        gts.append(gt)
                    for gg in range(GPC):
                        g = c * GPC + gg
                        for cor in range(4):
                            w_ = wc[cor][:, k, g : g + 1]
                            if k == 0 and cor == 0:
                                TS(xal[:, g, :], gts[cor][:, gg, :], w_, None, ALU.mult)
                            else:
                                STT(xal[:, g, :], gts[cor][:, gg, :], w_,
                                    xal[:, g, :], ALU.mult, ALU.add)
                # after all taps of chunk: add al_b, store band
                for gg in range(GPC):
                    g = c * GPC + gg
                    TT(out=xal[:, g, :], in0=xal[:, g, :], in1=albr[:], op=ALU.add)
                nc.sync.dma_start(
                    xalt[65 + c * 512 : 65 + (c + 1) * 512, :].rearrange(
                        "(a p) o -> p a o", p=P
                    ),
                    xal[:, c * GPC : (c + 1) * GPC, :],
                )

            # ---------------- P6: CARAFE ----------------
            for c in range(8):
                for k in range(9):
                    dy, dx = taps[k]
                    s = dy * 64 + dx
                    ct = cdp.tile([P, 4, CO], f16, tag="cd", name=f"cd_{c}_{k}")
                    nc.sync.dma_start(
                        ct[:],
                        xalt[65 + c * 512 + s : 65 + (c + 1) * 512 + s, :].rearrange(
                            "(a p) o -> p a o", p=P
                        ),
                    )
                    for gg in range(4):
                        g = c * 4 + gg
                        kw = keff[:, g, k : k + 1]
                        if k == 0:
                            TS(xdn[:, g, :], ct[:, gg, :], kw, None, ALU.mult)
                        else:
                            STT(xdn[:, g, :], ct[:, gg, :], kw,
                                xdn[:, g, :], ALU.mult, ALU.add)
                nc.sync.dma_start(
                    xdt[c * 512 : (c + 1) * 512, :].rearrange("(a p) o -> p a o", p=P),
                    xdn[:, c * 4 : (c + 1) * 4, :],
                )

            # ---------------- P7: xbar transposes back to ck-part ----------------
            xalp = pp.tile([P, 2, PADHW], f16, tag="xcat", name="xalp")
            xdp_ = pp.tile([P, 2, PADHW], f16, tag="wcf", name="xdp_")
            nc.gpsimd.memset(xalp[:], 0)
            nc.gpsimd.memset(xdp_[:], 0)
            for (dst, srcdram, r0) in ((xalp, xalt, 65), (xdp_, xdt, 0)):
                for cit in range(2):
                    for rh in range(2):
                        stg = pp.tile([P, 2048], f16, tag="zeros",
                                      name=f"stg_{r0}_{cit}_{rh}")
                        nc.sync.dma_start_transpose(
                            stg[:],
                            srcdram[r0 + 2048 * rh : r0 + 2048 * (rh + 1),
                                    cit * 128 : (cit + 1) * 128],
                        )
                        nc.sync.dma_start(
                            interior(dst, cit)[:, 1 + 32 * rh : 1 + 32 * (rh + 1), 1:65],
                            stg[:].rearrange("p (a b) -> p a b", a=32),
                        )

            # ---------------- P8: fo conv ----------------
            cat2 = [(xoff, 0), (xoff, 1), (xalp, 0), (xalp, 1), (xdp_, 0), (xdp_, 1)]
            for cot in range(2):
                for c in range(NCHUNK):
                    ps = pmm.tile([P, 512], f32, tag="pmm")
                    n = 0
                    for t in range(9):
                        dy, dx = taps[t]
                        for cit in range(6):
                            src, ci = cat2[cit]
                            j = ((t * 6 + cit) * 2 + cot) * P
                            nc.tensor.matmul(
                                ps[:],
                                lhsT=wfo[:, j : j + P],
                                rhs=rhs_ap(src, ci, c * 8, dy, dx),
                                start=(n == 0),
                                stop=(n == 53),
                            )
                            n += 1
                    ost = s32p.tile([P, 512], f32, tag="st32", name=f"ost_{cot}_{c}")
                    nc.scalar.activation(
                        ost[:], ps[:], ACT.Relu, bias=bfo[:, cot : cot + 1],
                    )
                    rneg2 = s32p.tile([P, 512], f32, tag="st32",
                                      name=f"rnfo_{cot}_{c}")
                    nc.scalar.activation(
                        rneg2[:], ps[:], ACT.Relu,
                        bias=bfon[:, cot : cot + 1], scale=-1.0,
                    )
                    nc.vector.scalar_tensor_tensor(
                        ost[:], rneg2[:], -0.1, ost[:], ALU.mult, ALU.add
                    )
                    nc.sync.dma_start(out_d[cot, :, c * 512 : (c + 1) * 512], ost[:])

            # ---------------- debug dumps ----------------
            if debug:
                nc.sync.dma_start(dbg["okT"][:, :],
                                  okT[:].rearrange("p g c -> p (g c)"))
                nc.sync.dma_start(dbg["kern"][:, :],
                                  keff[:].rearrange("p g k -> p (g k)"))
                nc.sync.dma_start(dbg["xal"][:, :],
                                  xal[:].rearrange("p g o -> p (g o)"))
                nc.sync.dma_start(dbg["xd"][:, :],
                                  xdn[:].rearrange("p g o -> p (g o)"))
                for ci in range(2):
                    nc.sync.dma_start(
                        dbg["xoff"][ci, :, :].rearrange("p (a b) -> p a b", a=64),
                        rhs_ap(xoff, ci, 0, 0, 0, nr=64),
                    )
                nc.sync.dma_start(dbg["idx"][:, :],
                                  idxf[:].rearrange("p k g -> p (k g)"))
                nc.sync.dma_start(dbg["wc0"][:, :],
                                  wc[0][:].rearrange("p k g -> p (k g)"))
                nc.sync.dma_start(dbg["wt"][:, :],
                                  WT[:].rearrange("p k s -> p (k s)"))

    nc.compile()
    return nc


def pack_inputs(inputs):
    """Host-side prep: per-core in_maps from full inputs."""
    f = np.float16
    X_O = np.asarray(inputs["X_O"], np.float32)
    X_in = np.asarray(inputs["X_in"], np.float32)
    B = X_O.shape[0]

    def conv_w(w, s=None):
        w = np.asarray(w, np.float32)
        if s is not None:
            w = w * np.asarray(s, np.float32)[:, None, None, None]
        return w

    cf_w = conv_w(inputs["cf_w"], inputs["cf_s"])
    off_w = conv_w(inputs["off_w"])
    al_w = conv_w(inputs["al_w"])
    e1_w = conv_w(inputs["e1_w"], inputs["e1_s"])
    e2_w = conv_w(inputs["e2_w"], inputs["e2_s"])
    fo_w = conv_w(inputs["fo_w"], inputs["fo_s"])

    # w_cf: [p, t, cit, cot, co] ; w[o, c, ky, kx], c = cit*128+p, o = cot*128+co
    w = cf_w.reshape(2, P, 4, P, 9)  # [cot, co, cit, p, t]
    w_cf = np.ascontiguousarray(w.transpose(3, 4, 2, 0, 1)).reshape(P, -1).astype(f)
    w = al_w.reshape(CO, 2, P, 9)  # [o, cit, p, t]
    w_al = np.ascontiguousarray(w.transpose(2, 1, 3, 0)).reshape(P, -1).astype(f)
    w = off_w.reshape(18, 2, P, 9)
    w_off = np.ascontiguousarray(w.transpose(2, 3, 1, 0)).reshape(P, -1).astype(f)
    w = e1_w.reshape(CMID, 2, P, 9)
    w_e1 = np.ascontiguousarray(w.transpose(2, 3, 1, 0)).reshape(P, -1).astype(f)
    w = e2_w.reshape(9, CMID, 9)
    w_e2 = np.ascontiguousarray(w.transpose(1, 2, 0)).reshape(CMID, -1).astype(f)
    w = fo_w.reshape(2, P, 6, P, 9)
    w_fo = np.ascontiguousarray(w.transpose(3, 4, 2, 0, 1)).reshape(P, -1).astype(f)

    b_cf = np.asarray(inputs["cf_sh"], np.float32).reshape(2, P).T.copy()
    b_off = np.asarray(inputs["off_b"], np.float32).reshape(18, 1)
    b_e1 = np.asarray(inputs["e1_sh"], np.float32).reshape(CMID, 1)
    b_e2 = np.asarray(inputs["e2_sh"], np.float32).reshape(9, 1)
    b_fo = np.asarray(inputs["fo_sh"], np.float32).reshape(2, P).T.copy()
    b_cf_n = -b_cf
    b_fo_n = -b_fo
    alb_rep = np.broadcast_to(
        np.asarray(inputs["al_b"], np.float32), (P, CO)
    ).astype(f).copy()

    # constants: px = g*128 + p ; y = px//64, x = px%64
    pxs = np.arange(HW)
    ppx = pxs.reshape(NG, P)  # [g, p]
    yy = (ppx // 64).T  # [p, g]
    xx = (ppx % 64).T
    ki = np.array([t // 3 - 1 for t in range(9)])
    kj = np.array([t % 3 - 1 for t in range(9)])
    gkiy = (yy[:, None, :] + ki[None, :, None] - 0.5).astype(np.float16).reshape(P, -1)
    gkjx = (xx[:, None, :] + kj[None, :, None] - 0.5).astype(np.float16).reshape(P, -1)
    vy = (yy[:, None, :] + ki[None, :, None] >= 0) & (
        yy[:, None, :] + ki[None, :, None] <= 63
    )
    vx = (xx[:, None, :] + kj[None, :, None] >= 0) & (
        xx[:, None, :] + kj[None, :, None] <= 63
    )
    vmask = (vy & vx).transpose(0, 2, 1).astype(f).reshape(P, -1)  # [p, g, k]

    shared = dict(
        w_cf=w_cf, w_al=w_al, w_off=w_off, w_e1=w_e1, w_e2=w_e2, w_fo=w_fo,
        b_cf=b_cf, b_off=b_off, b_e1=b_e1, b_e2=b_e2, b_fo=b_fo,
        b_cf_n=b_cf_n, b_fo_n=b_fo_n,
        alb_rep=alb_rep, vmask=vmask, gkiy=gkiy, gkjx=gkjx,
    )
    in_maps = []
    for b in range(B):
        m = dict(shared)
        m["xo"] = X_O[b].reshape(2, P, HW).astype(f)
        m["xi"] = X_in[b].reshape(2, P, HW).astype(f)
        in_maps.append(m)
    return in_maps


def kernel(**inputs):
    from concourse.bass_utils import run_bass_kernel_spmd

    if "nc" not in _CACHE:
        _CACHE["nc"] = build_kernel()
    nc = _CACHE["nc"]
    in_maps = pack_inputs(inputs)
    B = len(in_maps)
    res = run_bass_kernel_spmd(nc, in_maps, core_ids=list(range(B)))
    outs = [
        res.results[b]["out"].reshape(CO, H, W).astype(np.float32) for b in range(B)
    ]
    return np.stack(outs, axis=0)



# revision 32
# speedup vs baseline: 9.0051x; 9.0051x over previous
"""Trainium2 Bass kernel for nn_CSACMRFusion (deformable-conv + CARAFE fusion).

Self-contained: accepts FULL unsharded inputs, shards batch across 8 cores
(1 sample/core), runs one Bass/Tile kernel per core via run_bass_kernel_spmd,
gathers the full output.

Per-core pipeline (all on-chip math fp16, PSUM accumulate f32):
  P1  cf conv (classic matmul)  -> X_off (ck-part, padded)  [inputs host-padded]
  P3  e1+off stacked conv, e2 conv -> PE-transpose -> offsets/kern px-part
  P4  softmax + deform index/weight math (px-part) -> WT int16 indices
  P2  Y_k = al_w[:,:,k] @ X_in  (swapped matmuls) -> per-tap Y^T planes in DRAM
  P5  paired-corner dma_gathers (elem=512 spans x0,x0+1; two calls y0/y1);
      bilinear MACs split BY PIXEL GROUP: g0-2 Pool STT, g3-5 DVE STT,
      g6-7 Act(mult)+DVE(add); one-tap gather prefetch pipeline
  P6  CARAFE: shifted DRAM reads of X_align^T + kern-weighted MACs (mixed)
  P7  PE-transposes move X_align / X_down into ck-part padded tiles
  P8  fo conv (classic) + LeakyReLU -> out   (chunk-pipelined under P5/P6)
"""

import numpy as np

P = 128
H = W = 64
HW = 4096
NG = 32          # px groups: px = g*128 + p
CI = 256
CO = 256
CMID = 64
K9 = 9
PADW = 66
PADHW = PADW * PADW  # 4356
NCHUNK = 8       # spatial chunks of 8 rows = 512 px for classic convs
MAGIC = 12582912.0  # 3 * 2**22, f32 round-to-int magic
# per-superchunk gather-window start rows (px rows 16s-17 .. 16s+32)
LO_S = [64 * max(0, 16 * s - 17) for s in range(4)]

# engine pipe per local pixel-group j (0..7) inside a 1024-px superchunk
#   'VP' DVE TS-mult + Pool TT-add, 'V' DVE STT chain, 'A' Act mult + DVE TT
# (Pool cannot run the fused STT op or touch PSUM on real HW)
G_PIPE = ("VP", "VP", "V", "V", "A", "A", "A", "A")
# CARAFE chain engine per local g in a 512-px chunk
CAR_PATTERN = ("V", "A", "V", "A")

_CACHE = {}
USE_LRELU = False


def _taps():
    return [(k // 3 - 1, k % 3 - 1) for k in range(9)]


def build_kernel(debug=False):
    import concourse.bass as bass
    import concourse.tile as tile
    from concourse import bacc, mybir
    from concourse.ap import AP as APc
    from concourse.masks import make_identity

    f16 = mybir.dt.float16
    f32 = mybir.dt.float32
    i16 = mybir.dt.int16
    ALU = mybir.AluOpType
    ACT = mybir.ActivationFunctionType

    nc = bacc.Bacc("TRN2", target_bir_lowering=False, debug=False, num_devices=8)

    # ---------------- DRAM I/O ----------------
    xo_d = nc.dram_tensor("xo", [P, 2 * PADHW], f16, kind="ExternalInput")
    xi_d = nc.dram_tensor("xi", [P, 2 * PADHW], f16, kind="ExternalInput")
    wcf_d = nc.dram_tensor("w_cf", [P, 9 * 4 * 2 * P], f16, kind="ExternalInput")
    wal_d = nc.dram_tensor("w_al", [P, 2 * 9 * CO], f16, kind="ExternalInput")
    weo_d = nc.dram_tensor("w_eo", [P, 9 * 2 * 82], f16, kind="ExternalInput")
    we2_d = nc.dram_tensor("w_e2", [CMID, 9 * 9], f16, kind="ExternalInput")
    wfo_d = nc.dram_tensor("w_fo", [P, 9 * 6 * 2 * P], f16, kind="ExternalInput")
    bcf_d = nc.dram_tensor("b_cf", [P, 2], f32, kind="ExternalInput")
    beo_d = nc.dram_tensor("b_eo", [82, 1], f32, kind="ExternalInput")
    be2_d = nc.dram_tensor("b_e2", [9, 1], f32, kind="ExternalInput")
    bfo_d = nc.dram_tensor("b_fo", [P, 2], f32, kind="ExternalInput")
    bcfn_d = nc.dram_tensor("b_cf_n", [P, 2], f32, kind="ExternalInput")
    bfon_d = nc.dram_tensor("b_fo_n", [P, 2], f32, kind="ExternalInput")
    alb_d = nc.dram_tensor("alb_rep", [P, CO], f16, kind="ExternalInput")
    vmask_d = nc.dram_tensor("vmask", [P, NG * 9], f16, kind="ExternalInput")
    gkiy_d = nc.dram_tensor("gkiy", [P, 9 * NG], f16, kind="ExternalInput")
    gkjx_d = nc.dram_tensor("gkjx", [P, 9 * NG], f16, kind="ExternalInput")
    out_d = nc.dram_tensor("out", [2, P, HW], f32, kind="ExternalOutput")

    taps = _taps()

    with tile.TileContext(nc) as tc:
        with (
            tc.tile_pool(name="persist", bufs=1) as pp,
            tc.tile_pool(name="dram", bufs=1, space="DRAM") as dp,
        ):
            # ---------------- persistent SBUF ----------------
            xcat = pp.tile([P, 4, PADHW], f16, tag="xcat")
            xoff = pp.tile([P, 2, PADHW], f16, tag="xoff")
            wfo = pp.tile([P, 9 * 6 * 2 * P], f16, tag="wfo")
            bcf = pp.tile([P, 2], f32, tag="bcf")
            beo = pp.tile([82, 1], f32, tag="beo")
            be2 = pp.tile([9, 1], f32, tag="be2")
            bfo = pp.tile([P, 2], f32, tag="bfo")
            bcfn = pp.tile([P, 2], f32, tag="bcfn")
            bfon = pp.tile([P, 2], f32, tag="bfon")
            albr = pp.tile([P, CO], f16, tag="albr")
            ident = pp.tile([P, P], f32, tag="ident")
            identh = pp.tile([P, P], f16, tag="identh")
            okT = pp.tile([P, NG, 27], f32, tag="okT")
            wal = pp.tile([P, 2 * 9 * CO], f16, tag="wal")
            zeros = pp.tile([P, CO], f16, tag="zeros")
            WT = pp.tile([P, 9, 256], i16, tag="WT")
            keff = pp.tile([P, NG, 9], f32, tag="keff")
            wc = [pp.tile([P, 9, NG], f32, tag=f"wc{i}", name=f"wc{i}")
                  for i in range(4)]

            # ---------------- DRAM scratch ----------------
            yt = dp.tile([9, 4226, CO], f16, tag="yt")
            xalt = dp.tile([4226, CO], f16, tag="xalt")

            def interior(padtile, ci):
                return padtile[:, ci, :].rearrange("p (h w) -> p h w", h=PADW)

            def rhs_ap(padtile, ci, r0, dy, dx, nr=8):
                # [Ppart, nr rows, 64] shifted view inside padded image
                v = interior(padtile, ci)
                return v[:, 1 + r0 + dy : 1 + r0 + nr + dy, 1 + dx : 65 + dx]

            def zero_margins(padtile, ci, npart=P):
                v = interior(padtile, ci)[0:npart]
                nc.gpsimd.memset(v[:, 0, :], 0)
                nc.gpsimd.memset(v[:, 65, :], 0)
                nc.gpsimd.memset(v[:, 1:65, 0:1], 0)
                nc.gpsimd.memset(v[:, 1:65, 65:66], 0)

            STT_V = nc.vector.scalar_tensor_tensor
            STT_P = nc.gpsimd.scalar_tensor_tensor
            TT_V = nc.vector.tensor_tensor
            TT_P = nc.gpsimd.tensor_tensor
            TS_V = nc.vector.tensor_scalar
            TS_P = nc.gpsimd.tensor_scalar

            # ---------------- P0: loads (inputs pre-padded on host) --------
            # DMA issue order matters: P1 needs xcat + wcf first.
            nc.sync.dma_start(
                xcat[:, 0:2, :].rearrange("p a b -> p (a b)"), xo_d[:])
            nc.sync.dma_start(
                xcat[:, 2:4, :].rearrange("p a b -> p (a b)"), xi_d[:])
            for ci in range(2):
                zero_margins(xoff, ci)
            make_identity(nc, ident[:])
            nc.vector.tensor_copy(identh[:], ident[:])
            nc.vector.memset(zeros[:], 0)

            def load_rest():
                nc.sync.dma_start(wfo[:], wfo_d[:])
                for sb, dr in ((bcf, bcf_d), (beo, beo_d), (be2, be2_d),
                               (bfo, bfo_d), (albr, alb_d),
                               (bcfn, bcfn_d), (bfon, bfon_d)):
                    nc.sync.dma_start(sb[:], dr[:])
                # zero margins of DRAM scratch
                for k in range(9):
                    nc.sync.dma_start(yt[k, 0:65, :], zeros[0:65, :])
                    nc.sync.dma_start(yt[k, 4161:4226, :], zeros[0:65, :])
                nc.sync.dma_start(xalt[0:65, :], zeros[0:65, 0:CO])
                nc.sync.dma_start(xalt[4161:4226, :], zeros[0:65, 0:CO])

            _ysp_cm = tc.tile_pool(name="ystage", bufs=2)
            ysp = _ysp_cm.__enter__()
            _pyy_cm = tc.tile_pool(name="py", bufs=4, space="PSUM")
            pyy = _pyy_cm.__enter__()

            # P2 band emitter: Y_k swapped matmuls -> per-tap yt planes
            ysegs = [(0, 512), (512, 512), (1024, 512), (1536, 512),
                     (2048, 256)]

            def emit_p2_band(g, late=False):
                yst = ysp.tile([P, 9, CO], f16, tag="yst", name=f"yst{g}")
                lsts = []
                for cit in range(2):
                    lst = ysp.tile([P, 128], f16, tag=f"lst{cit}",
                                   name=f"lst{cit}_{g}")
                    nc.vector.tensor_copy(
                        lst[:].rearrange("p (a b) -> p a b", a=2),
                        rhs_ap(xcat, 2 + cit, g * 2, 0, 0, nr=2),
                    )
                    lsts.append(lst)
                ystf = yst[:].rearrange("p k c -> p (k c)")
                for si, (o0, nn) in enumerate(ysegs):
                    ps = pyy.tile([P, 512], f32, tag="pyq",
                                  name=f"pyq{g}_{si}")
                    for cit in range(2):
                        nc.tensor.matmul(
                            ps[:, 0:nn],
                            lhsT=lsts[cit][:],
                            rhs=wal[:, cit * 2304 + o0 : cit * 2304 + o0 + nn],
                            start=(cit == 0),
                            stop=(cit == 1),
                        )
                    if late and si % 2 == 1:
                        nc.vector.tensor_copy(ystf[:, o0 : o0 + nn], ps[:, 0:nn])
                    else:
                        nc.scalar.activation(
                            ystf[:, o0 : o0 + nn], ps[:, 0:nn], ACT.Copy
                        )
                nc.sync.dma_start(
                    yt[:, 65 + g * 128 : 65 + (g + 1) * 128, :]
                    .transpose([1, 0, 2]),
                    yst[:],
                )

            with (
                tc.tile_pool(name="wearly", bufs=1) as wp,
                tc.tile_pool(name="stage32", bufs=2) as s32p,
                tc.tile_pool(name="pmain", bufs=2, space="PSUM") as pmm,
                tc.tile_pool(name="psmall", bufs=2, space="PSUM") as psm,
            ):
                wcf = wp.tile([P, 9 * 4 * 2 * P], f16, tag="wcf")
                weo = wp.tile([P, 9 * 2 * 82], f16, tag="weo")
                we2 = wp.tile([CMID, 9 * 9], f16, tag="we2")
                e1p = wp.tile([CMID, PADHW], f16, tag="e1p")
                nc.sync.dma_start(wcf[:], wcf_d[:])
                nc.sync.dma_start(weo[:], weo_d[:])
                nc.sync.dma_start(we2[:], we2_d[:])
                nc.sync.dma_start(wal[:], wal_d[:])
                load_rest()
                e1i = e1p[:].rearrange("p (h w) -> p h w", h=PADW)
                nc.gpsimd.memset(e1i[:, 0, :], 0)
                nc.gpsimd.memset(e1i[:, 65, :], 0)
                nc.gpsimd.memset(e1i[:, 1:65, 0:1], 0)
                nc.gpsimd.memset(e1i[:, 1:65, 65:66], 0)

                # ---------------- P1: cf conv ----------------
                for cot in range(2):
                    for c in range(NCHUNK):
                        ps = pmm.tile([P, 512], f32, tag="pmm")
                        n = 0
                        for t in range(9):
                            dy, dx = taps[t]
                            for cit in range(4):
                                j = ((t * 4 + cit) * 2 + cot) * P
                                nc.tensor.matmul(
                                    ps[:],
                                    lhsT=wcf[:, j : j + P],
                                    rhs=rhs_ap(xcat, cit, c * 8, dy, dx),
                                    start=(n == 0),
                                    stop=(n == 35),
                                )
                                n += 1
                        dstv = rhs_ap(xoff, cot, c * 8, 0, 0)
                        if USE_LRELU:
                            nc.scalar.activation(
                                dstv, ps[:].rearrange("p (a b) -> p a b", a=8),
                                ACT.Lrelu, bias=bcf[:, cot : cot + 1], alpha=0.1,
                            )
                        else:
                            nc.scalar.activation(
                                dstv, ps[:].rearrange("p (a b) -> p a b", a=8),
                                ACT.Relu, bias=bcf[:, cot : cot + 1],
                            )
                            rneg = s32p.tile([P, 512], f16, tag="st32",
                                             name=f"rncf_{cot}_{c}")
                            nc.scalar.activation(
                                rneg[:], ps[:], ACT.Relu,
                                bias=bcfn[:, cot : cot + 1], scale=-1.0,
                            )
                            STT_V(dstv, rneg[:].rearrange("p (a b) -> p a b", a=8),
                                  -0.1, dstv, ALU.mult, ALU.add)

                # ---------------- P3a: e1+off stacked conv ----------------
                for c in range(NCHUNK):
                    ps = psm.tile([82, 512], f32, tag="sm", name=f"peo_{c}")
                    n = 0
                    for t in range(9):
                        dy, dx = taps[t]
                        for cit in range(2):
                            j = (t * 2 + cit) * 82
                            nc.tensor.matmul(
                                ps[:],
                                lhsT=weo[:, j : j + 82],
                                rhs=rhs_ap(xoff, cit, c * 8, dy, dx),
                                start=(n == 0),
                                stop=(n == 17),
                            )
                            n += 1
                    nc.scalar.activation(
                        e1i[0:CMID, 1 + c * 8 : 9 + c * 8, 1:65],
                        ps[0:CMID, :].rearrange("p (a b) -> p a b", a=8),
                        ACT.Identity,
                        bias=beo[0:CMID, 0:1],
                    )
                    st = s32p.tile([18, 512], f32, tag="st32", name=f"sto_{c}")
                    nc.vector.tensor_scalar(
                        st[:], ps[CMID:82, :], beo[CMID:82, 0:1], -16.0,
                        ALU.add, ALU.max,
                    )
                    nc.vector.tensor_scalar(
                        st[:], st[:], 16.0, None, ALU.min
                    )
                    for q in range(4):
                        pt = psm.tile([P, 32], f32, tag="sm", name=f"ptr_{c}_{q}")
                        nc.tensor.transpose(
                            pt[:, 0:18], st[:, q * 128 : (q + 1) * 128],
                            ident[0:18, 0:18],
                        )
                        nc.vector.tensor_copy(okT[:, c * 4 + q, 0:18], pt[:, 0:18])

                # ---------------- P3b: e2 conv, transpose to px-part ----------
                for c in range(NCHUNK):
                    pse = psm.tile([9, 512], f32, tag="sm", name=f"pe2_{c}")
                    for t in range(9):
                        dy, dx = taps[t]
                        nc.tensor.matmul(
                            pse[:],
                            lhsT=we2[:, t * 9 : (t + 1) * 9],
                            rhs=e1i[0:CMID, 1 + c * 8 + dy : 9 + c * 8 + dy,
                                    1 + dx : 65 + dx],
                            start=(t == 0),
                            stop=(t == 8),
                        )
                    stk = s32p.tile([9, 512], f32, tag="st32", name=f"stk_{c}")
                    nc.scalar.activation(
                        stk[:], pse[:], ACT.Identity, bias=be2[:, 0:1]
                    )
                    for q in range(4):
                        pt2 = psm.tile([P, 32], f32, tag="sm", name=f"ptk_{c}_{q}")
                        nc.tensor.transpose(
                            pt2[:, 0:9], stk[:, q * 128 : (q + 1) * 128],
                            ident[0:9, 0:9],
                        )
                        nc.vector.tensor_copy(okT[:, c * 4 + q, 18:27], pt2[:, 0:9])

                # ---------------- P4: softmax + deform index math ------------
                with tc.tile_pool(name="dmath", bufs=1) as dmp:
                    vmask = dmp.tile([P, NG, 9], f16, tag="vmask")
                    gkiy = dmp.tile([P, 9, NG], f16, tag="gkiy")
                    gkjx = dmp.tile([P, 9, NG], f16, tag="gkjx")
                    nc.sync.dma_start(
                        vmask[:].rearrange("p g k -> p (g k)"), vmask_d[:])
                    nc.sync.dma_start(
                        gkiy[:].rearrange("p k g -> p (k g)"), gkiy_d[:])
                    nc.sync.dma_start(
                        gkjx[:].rearrange("p k g -> p (k g)"), gkjx_d[:])
                    expt = dmp.tile([P, NG, 9], f32, tag="expt")
                    den = dmp.tile([P, NG, 1], f32, tag="den")
                    rec = dmp.tile([P, NG, 1], f32, tag="rec")
                    ys = dmp.tile([P, 9, NG], f32, tag="ys")
                    xs = dmp.tile([P, 9, NG], f32, tag="xs")
                    y0f = dmp.tile([P, 9, NG], f32, tag="y0f")
                    x0f = dmp.tile([P, 9, NG], f32, tag="x0f")
                    fy = dmp.tile([P, 9, NG], f32, tag="fy")
                    fx = dmp.tile([P, 9, NG], f32, tag="fx")
                    tmpa = dmp.tile([P, 9, NG], f32, tag="tmpa")
                    tmpb = dmp.tile([P, 9, NG], f32, tag="tmpb")
                    wy0 = dmp.tile([P, 9, NG], f32, tag="wy0")
                    wy1 = dmp.tile([P, 9, NG], f32, tag="wy1")
                    wx0 = dmp.tile([P, 9, NG], f32, tag="wx0")
                    wx1 = dmp.tile([P, 9, NG], f32, tag="wx1")
                    idxf = dmp.tile([P, 9, NG], f32, tag="idxf")
                    tsb = [dmp.tile([P, P], f32, tag=f"tsb{j}", name=f"tsb{j}")
                           for j in range(3)]

                    nc.scalar.activation(expt[:], okT[:, :, 18:27], ACT.Exp)
                    nc.vector.tensor_reduce(den[:], expt[:],
                                            axis=mybir.AxisListType.X, op=ALU.add)
                    nc.vector.reciprocal(rec[:], den[:])
                    for g in range(NG):
                        nc.vector.tensor_scalar(
                            keff[:, g, :], expt[:, g, :], rec[:, g, 0:1],
                            None, ALU.mult
                        )
                    nc.vector.tensor_tensor(
                        out=keff[:], in0=keff[:], in1=vmask[:], op=ALU.mult
                    )

                    # offsets: okT ch 2k = dy_k, 2k+1 = dx_k ; view as [p, k, g]
                    okv = okT[:].rearrange("p g c -> p c g")
                    dys = okv[:, 0:18:2, :]
                    dxs = okv[:, 1:18:2, :]
                    fl = lambda t_: t_[:]
                    TT_V(out=fl(ys), in0=dys, in1=fl(gkiy), op=ALU.add)
                    TT_V(out=fl(xs), in0=dxs, in1=fl(gkjx), op=ALU.add)
                    TS_V(fl(y0f), fl(ys), MAGIC, MAGIC, ALU.add, ALU.subtract)
                    TS_V(fl(x0f), fl(xs), MAGIC, MAGIC, ALU.add, ALU.subtract)
                    STT_V(fl(fy), fl(ys), 0.5, fl(y0f), ALU.add, ALU.subtract)
                    STT_V(fl(fx), fl(xs), 0.5, fl(x0f), ALU.add, ALU.subtract)

                    def valid(dst, src, lo, hi):
                        TS_V(fl(tmpa), fl(src), lo, None, ALU.is_ge)
                        TS_V(fl(tmpb), fl(src), hi, None, ALU.is_le)
                        TT_V(out=fl(dst), in0=fl(tmpa), in1=fl(tmpb), op=ALU.mult)

                    valid(wy0, y0f, 0.0, 63.0)
                    valid(wy1, y0f, -1.0, 62.0)
                    valid(wx0, x0f, 0.0, 63.0)
                    valid(wx1, x0f, -1.0, 62.0)
                    TS_V(fl(tmpa), fl(fy), -1.0, 1.0, ALU.mult, ALU.add)  # 1-fy
                    TT_V(out=fl(wy0), in0=fl(wy0), in1=fl(tmpa), op=ALU.mult)
                    TT_V(out=fl(wy1), in0=fl(wy1), in1=fl(fy), op=ALU.mult)
                    TS_V(fl(tmpa), fl(fx), -1.0, 1.0, ALU.mult, ALU.add)  # 1-fx
                    TT_V(out=fl(wx0), in0=fl(wx0), in1=fl(tmpa), op=ALU.mult)
                    TT_V(out=fl(wx1), in0=fl(wx1), in1=fl(fx), op=ALU.mult)
                    TT_V(out=fl(wc[0]), in0=fl(wy0), in1=fl(wx0), op=ALU.mult)
                    TT_V(out=fl(wc[1]), in0=fl(wy0), in1=fl(wx1), op=ALU.mult)
                    TT_V(out=fl(wc[2]), in0=fl(wy1), in1=fl(wx0), op=ALU.mult)
                    TT_V(out=fl(wc[3]), in0=fl(wy1), in1=fl(wx1), op=ALU.mult)
                    # base corner row: idx = 64*clip(y0+1, 0, 64) + clip(x0+1, 0, 64)
                    TS_V(fl(tmpa), fl(y0f), 1.0, 0.0, ALU.add, ALU.max)
                    TS_V(fl(tmpa), fl(tmpa), 64.0, None, ALU.min)
                    TS_V(fl(tmpb), fl(x0f), 1.0, 0.0, ALU.add, ALU.max)
                    TS_V(fl(tmpb), fl(tmpb), 64.0, None, ALU.min)
                    STT_V(fl(idxf), fl(tmpa), 64.0, fl(tmpb), ALU.mult, ALU.add)
                    # rebase indices per superchunk gather window
                    for s in range(1, 4):
                        if LO_S[s]:
                            TS_V(idxf[:, :, s * 8 : (s + 1) * 8],
                                 idxf[:, :, s * 8 : (s + 1) * 8],
                                 float(-LO_S[s]), None, ALU.add)

                    # ---------- P4b: wrap fold idx -> [16-rep, 9, 256] int16 --
                    chunks = [(0, 4), (4, 4), (8, 1)]
                    for j, (k0, nk) in enumerate(chunks):
                        pt1 = psm.tile([P, P], f32, tag="sm", name=f"pt1_{j}")
                        nc.tensor.transpose(
                            pt1[0 : nk * 32, 0:P],
                            idxf[:, k0 : k0 + nk, :].rearrange("p a b -> p (a b)"),
                            ident[:],
                        )
                        nc.vector.tensor_copy(tsb[j][0 : nk * 32, :],
                                              pt1[0 : nk * 32, :])
                    for j, (k0, nk) in enumerate(chunks):
                        for u in range(8):
                            pt2 = psm.tile([16, P], f32, tag="sm",
                                           name=f"pt2_{j}_{u}")
                            nc.tensor.transpose(
                                pt2[:, 0 : nk * 32],
                                tsb[j][0 : nk * 32, 16 * u : 16 * u + 16],
                                ident[0 : nk * 32, 0 : nk * 32],
                            )
                            nc.vector.tensor_copy(
                                WT[0:16, k0 : k0 + nk, u : u + 249 : 8],
                                pt2[:, 0 : nk * 32]
                            )
                    for r in range(1, 8):
                        nc.sync.dma_start(
                            WT[16 * r : 16 * r + 16, :, :], WT[0:16, :, :]
                        )

                # ---- P2 (part 1): bands needed by superchunk 0 gathers ----
                for g in range(18):
                    emit_p2_band(g)

            # ======== P5..P8 pipeline over 4 superchunks of 1024 px ========
            with (
                tc.tile_pool(name="gat", bufs=2) as gdp,
                tc.tile_pool(name="macacc", bufs=2) as map_,
                tc.tile_pool(name="atmp", bufs=8) as atp,
                tc.tile_pool(name="cdst", bufs=2) as cdp,
                tc.tile_pool(name="s32b", bufs=2) as s32b,
                tc.tile_pool(name="pmain2", bufs=2, space="PSUM") as pm2,
                tc.tile_pool(name="ptr2", bufs=2, space="PSUM") as ptp,
            ):
                alck = pp.tile([P, 4, PADHW], f16, tag="xcat", name="alck")
                for ci in range(4):
                    zero_margins(alck, ci)

                def px2ck(src_slc, dsttile, plane, g, nm, eng="A"):
                    # [128 px, 128 ch] slice -> transpose -> interior rows 2g,2g+1
                    pt = ptp.tile([P, P], f16, tag="ptx", name=f"ptx_{nm}")
                    nc.tensor.transpose(pt[:], src_slc, identh[:])
                    dst = interior(dsttile, plane)[:, 1 + 2 * g : 3 + 2 * g, 1:65]
                    srcv = pt[:].rearrange("p (a b) -> p a b", a=2)
                    if eng == "A":
                        nc.scalar.activation(dst, srcv, ACT.Copy)
                    else:
                        nc.vector.tensor_copy(dst, srcv)

                def emit_carafe(c):
                    # X_down for chunk c (4 g's, 512 px) from shifted xalt reads
                    cdn = cdp.tile([P, 4, CO], f16, tag="cdn", name=f"cdn_{c}")
                    for k in range(9):
                        dy, dx = taps[k]
                        s = dy * 64 + dx
                        ct = cdp.tile([P, 4, CO], f16, tag="cd", name=f"cd_{c}_{k}")
                        nc.sync.dma_start(
                            ct[:],
                            xalt[65 + c * 512 + s : 65 + (c + 1) * 512 + s, :]
                            .rearrange("(a p) o -> p a o", p=P),
                        )
                        for gg in range(4):
                            g = c * 4 + gg
                            kw = keff[:, g, k : k + 1]
                            eng = CAR_PATTERN[gg % len(CAR_PATTERN)]
                            dst = cdn[:, gg, :]
                            src = ct[:, gg, :]
                            if eng == "VP":
                                at = atp.tile([P, CO], f16, tag="at",
                                              name=f"cvp_{c}_{k}_{gg}")
                                TS_V(at[:], src, kw, None, ALU.mult)
                                if k == 0:
                                    nc.gpsimd.tensor_copy(dst, at[:])
                                else:
                                    TT_P(out=dst, in0=dst, in1=at[:], op=ALU.add)
                            elif eng == "V":
                                if k == 0:
                                    TS_V(dst, src, kw, None, ALU.mult)
                                else:
                                    STT_V(dst, src, kw, dst, ALU.mult, ALU.add)
                            else:  # Act mult + DVE add
                                if k == 0:
                                    nc.scalar.activation(dst, src, ACT.Copy,
                                                         scale=kw)
                                else:
                                    at = atp.tile([P, CO], f16, tag="at",
                                                  name=f"cat_{c}_{k}_{gg}")
                                    nc.scalar.activation(at[:], src, ACT.Copy,
                                                         scale=kw)
                                    TT_V(out=dst, in0=dst, in1=at[:], op=ALU.add)
                    for gg in range(4):
                        for cb in range(2):
                            px2ck(cdn[:, gg, cb * 128 : (cb + 1) * 128],
                                  alck, 2 + cb, c * 4 + gg, f"d{c}_{gg}_{cb}",
                                  eng="A" if gg % 2 else "V")

                cat2 = [(xoff, 0), (xoff, 1), (alck, 0), (alck, 1),
                        (alck, 2), (alck, 3)]

                def emit_fo_chunk(c):
                    for cot in range(2):
                        ps = pm2.tile([P, 512], f32, tag="pmm2",
                                      name=f"pfo_{c}_{cot}")
                        n = 0
                        for t in range(9):
                            dy, dx = taps[t]
                            for cit in range(6):
                                src, ci = cat2[cit]
                                j = ((t * 6 + cit) * 2 + cot) * P
                                nc.tensor.matmul(
                                    ps[:],
                                    lhsT=wfo[:, j : j + P],
                                    rhs=rhs_ap(src, ci, c * 8, dy, dx),
                                    start=(n == 0),
                                    stop=(n == 53),
                                )
                                n += 1
                        ost = s32b.tile([P, 512], f32, tag="ost",
                                        name=f"ost_{cot}_{c}")
                        if USE_LRELU:
                            nc.scalar.activation(
                                ost[:], ps[:], ACT.Lrelu,
                                bias=bfo[:, cot : cot + 1], alpha=0.1,
                            )
                        else:
                            nc.scalar.activation(
                                ost[:], ps[:], ACT.Relu, bias=bfo[:, cot : cot + 1],
                            )
                            rneg2 = s32b.tile([P, 512], f32, tag="ost",
                                              name=f"rnfo_{cot}_{c}")
                            nc.scalar.activation(
                                rneg2[:], ps[:], ACT.Relu,
                                bias=bfon[:, cot : cot + 1], scale=-1.0,
                            )
                            STT_V(ost[:], rneg2[:], -0.1, ost[:], ALU.mult, ALU.add)
                        nc.sync.dma_start(
                            out_d[cot, :, c * 512 : (c + 1) * 512], ost[:]
                        )

                gsem = nc.alloc_semaphore("gat_sem")
                # superchunk gather windows (indices rebased by LO_S in P4):
                # W = max rebased idx + 1; window rows [lo+d, lo+d+W+1)
                IDX_MAX = [64 * min(64, 16 * s + 33) + 64 for s in range(4)]
                W_S = [IDX_MAX[s] - LO_S[s] + 1 for s in range(4)]

                def emit_gathers(s, k):
                    # Two preps per (s, k): y-corner pairs via d-offset base;
                    # in_ap windowed to rows [lo_s+d, lo_s+d+W_s) so the tile
                    # dep only covers the yt bands this superchunk reads.
                    gt = gdp.tile([P, 16, 512], f16, tag="gd",
                                  name=f"gd_{s}_{k}")
                    for yc, d in ((0, 0), (1, 64)):
                        v = yt[k, LO_S[s] + d : LO_S[s] + d + W_S[s], :]
                        v2 = APc(v.tensor, v.offset, [[256, W_S[s]], [1, 512]])
                        nc.gpsimd.dma_gather(
                            gt[:, 8 * yc : 8 * (yc + 1), :], v2,
                            WT[:, k, s * 64 : (s + 1) * 64],
                            1024, 1024, 512, elem_step=256,
                        )
                    return gt

                def emit_macs(s, k, gt, xal_s):
                    for j in range(8):
                        g = s * 8 + j
                        vv = [gt[:, j, 0:256], gt[:, j, 256:512],
                              gt[:, 8 + j, 0:256], gt[:, 8 + j, 256:512]]
                        pipe = G_PIPE[j]
                        dst = xal_s[:, j, :]
                        for cor in range(4):
                            w_ = wc[cor][:, k, g : g + 1]
                            first = k == 0 and cor == 0
                            if pipe == "V":
                                if first:
                                    STT_V(dst, vv[cor], w_, albr[:],
                                          ALU.mult, ALU.add)
                                else:
                                    STT_V(dst, vv[cor], w_, dst,
                                          ALU.mult, ALU.add)
                            elif pipe == "VP":
                                at = atp.tile([P, CO], f16, tag="at",
                                              name=f"vp_{s}_{k}_{j}_{cor}")
                                TS_V(at[:], vv[cor], w_, None, ALU.mult)
                                if first:
                                    TT_P(out=dst, in0=at[:], in1=albr[:],
                                         op=ALU.add)
                                else:
                                    TT_P(out=dst, in0=dst, in1=at[:],
                                         op=ALU.add)
                            else:
                                at = atp.tile([P, CO], f16, tag="at",
                                              name=f"at_{s}_{k}_{j}_{cor}")
                                nc.scalar.activation(at[:], vv[cor],
                                                     ACT.Copy, scale=w_)
                                if first:
                                    TT_V(out=dst, in0=at[:], in1=albr[:],
                                         op=ALU.add)
                                else:
                                    TT_V(out=dst, in0=dst, in1=at[:],
                                         op=ALU.add)

                P2_BANDS = {1: range(18, 26), 2: range(26, 32)}
                for s in range(4):
                    for g in P2_BANDS.get(s, ()):
                        emit_p2_band(g, late=True)
                    xal_s = map_.tile([P, 8, CO], f16, tag="xal", name=f"xal{s}")
                    gt = emit_gathers(s, 0)
                    for k in range(9):
                        gt_next = emit_gathers(s, k + 1) if k < 8 else None
                        emit_macs(s, k, gt, xal_s)
                        gt = gt_next
                    # store 2 bands of 512 px to xalt (for CARAFE shifts)
                    for h in range(2):
                        c = 2 * s + h
                        nc.sync.dma_start(
                            xalt[65 + c * 512 : 65 + (c + 1) * 512, :]
                            .rearrange("(a p) o -> p a o", p=P),
                            xal_s[:, h * 4 : (h + 1) * 4, :],
                        )
                    # PE-transpose X_align into ck-part padded tile
                    for j in range(8):
                        for cb in range(2):
                            px2ck(xal_s[:, j, cb * 128 : (cb + 1) * 128],
                                  alck, cb, s * 8 + j, f"a{s}_{j}_{cb}",
                                  eng="A" if j % 2 else "V")
                    # trailing CARAFE / fo conv
                    if s == 0:
                        emit_carafe(0)
                    else:
                        emit_carafe(2 * s - 1)
                        emit_carafe(2 * s)
                        emit_fo_chunk(2 * s - 2)
                        emit_fo_chunk(2 * s - 1)
                # tail
                emit_carafe(7)
                emit_fo_chunk(6)
                emit_fo_chunk(7)
            _ysp_cm.__exit__(None, None, None)
            _pyy_cm.__exit__(None, None, None)

    nc.compile()
    return nc


def pack_inputs(inputs):
    """Host-side prep: per-core in_maps from full inputs."""
    f = np.float16
    X_O = np.asarray(inputs["X_O"], np.float32)
    X_in = np.asarray(inputs["X_in"], np.float32)
    B = X_O.shape[0]

    def conv_w(w, s=None):
        w = np.asarray(w, np.float32)
        if s is not None:
            w = w * np.asarray(s, np.float32)[:, None, None, None]
        return w

    cf_w = conv_w(inputs["cf_w"], inputs["cf_s"])
    off_w = conv_w(inputs["off_w"])
    al_w = conv_w(inputs["al_w"])
    e1_w = conv_w(inputs["e1_w"], inputs["e1_s"])
    e2_w = conv_w(inputs["e2_w"], inputs["e2_s"])
    fo_w = conv_w(inputs["fo_w"], inputs["fo_s"])

    # w_cf: [p, t, cit, cot, co] ; w[o, c, ky, kx], c = cit*128+p, o = cot*128+co
    w = cf_w.reshape(2, P, 4, P, 9)  # [cot, co, cit, p, t]
    w_cf = np.ascontiguousarray(w.transpose(3, 4, 2, 0, 1)).reshape(P, -1).astype(f)
    w = al_w.reshape(CO, 2, P, 9)  # [o, cit, p, t]
    w_al = np.ascontiguousarray(w.transpose(2, 1, 3, 0)).reshape(P, -1).astype(f)
    # stacked e1 (64 out) + off (18 out): [p, t, cit, 82]
    w_e1 = e1_w.reshape(CMID, 2, P, 9).transpose(2, 3, 1, 0)  # [p, t, cit, 64]
    w_of = off_w.reshape(18, 2, P, 9).transpose(2, 3, 1, 0)   # [p, t, cit, 18]
    w_eo = np.concatenate([w_e1, w_of], axis=3)
    w_eo = np.ascontiguousarray(w_eo).reshape(P, -1).astype(f)
    w = e2_w.reshape(9, CMID, 9)
    w_e2 = np.ascontiguousarray(w.transpose(1, 2, 0)).reshape(CMID, -1).astype(f)
    w = fo_w.reshape(2, P, 6, P, 9)
    w_fo = np.ascontiguousarray(w.transpose(3, 4, 2, 0, 1)).reshape(P, -1).astype(f)

    b_cf = np.asarray(inputs["cf_sh"], np.float32).reshape(2, P).T.copy()
    b_eo = np.concatenate([
        np.asarray(inputs["e1_sh"], np.float32).reshape(CMID),
        np.asarray(inputs["off_b"], np.float32).reshape(18),
    ]).reshape(82, 1)
    b_e2 = np.asarray(inputs["e2_sh"], np.float32).reshape(9, 1)
    b_fo = np.asarray(inputs["fo_sh"], np.float32).reshape(2, P).T.copy()
    b_cf_n = -b_cf
    b_fo_n = -b_fo
    alb_rep = np.broadcast_to(
        np.asarray(inputs["al_b"], np.float32), (P, CO)
    ).astype(f).copy()

    # constants: px = g*128 + p ; y = px//64, x = px%64
    pxs = np.arange(HW)
    ppx = pxs.reshape(NG, P)  # [g, p]
    yy = (ppx // 64).T  # [p, g]
    xx = (ppx % 64).T
    ki = np.array([t // 3 - 1 for t in range(9)])
    kj = np.array([t % 3 - 1 for t in range(9)])
    gkiy = (yy[:, None, :] + ki[None, :, None] - 0.5).astype(np.float16).reshape(P, -1)
    gkjx = (xx[:, None, :] + kj[None, :, None] - 0.5).astype(np.float16).reshape(P, -1)
    vy = (yy[:, None, :] + ki[None, :, None] >= 0) & (
        yy[:, None, :] + ki[None, :, None] <= 63
    )
    vx = (xx[:, None, :] + kj[None, :, None] >= 0) & (
        xx[:, None, :] + kj[None, :, None] <= 63
    )
    vmask = (vy & vx).transpose(0, 2, 1).astype(f).reshape(P, -1)  # [p, g, k]

    def pad_planes(x):
        # [2, 128, 4096] -> host-padded [128, 2, 66, 66] -> [128, 2*4356]
        out = np.zeros((2, P, PADW, PADW), np.float32)
        out[:, :, 1:65, 1:65] = x.reshape(2, P, 64, 64)
        return np.ascontiguousarray(
            out.transpose(1, 0, 2, 3).reshape(P, -1)).astype(f)

    shared = dict(
        w_cf=w_cf, w_al=w_al, w_eo=w_eo, w_e2=w_e2, w_fo=w_fo,
        b_cf=b_cf, b_eo=b_eo, b_e2=b_e2, b_fo=b_fo,
        b_cf_n=b_cf_n, b_fo_n=b_fo_n,
        alb_rep=alb_rep, vmask=vmask, gkiy=gkiy, gkjx=gkjx,
    )
    in_maps = []
    for b in range(B):
        m = dict(shared)
        m["xo"] = pad_planes(X_O[b].reshape(2, P, HW))
        m["xi"] = pad_planes(X_in[b].reshape(2, P, HW))
        in_maps.append(m)
    return in_maps


def kernel(**inputs):
    from concourse.bass_utils import run_bass_kernel_spmd

    if "nc" not in _CACHE:
        _CACHE["nc"] = build_kernel()
    nc = _CACHE["nc"]
    in_maps = pack_inputs(inputs)
    B = len(in_maps)
    res = run_bass_kernel_spmd(nc, in_maps, core_ids=list(range(B)))
    outs = [
        res.results[b]["out"].reshape(CO, H, W).astype(np.float32) for b in range(B)
    ]
    return np.stack(outs, axis=0)


# revision 39
# speedup vs baseline: 10.5241x; 1.1687x over previous
"""Trainium2 Bass kernel for nn_CSACMRFusion (deformable-conv + CARAFE fusion).

Self-contained: accepts FULL unsharded inputs, shards batch across 8 cores
(1 sample/core), runs one Bass/Tile kernel per core via run_bass_kernel_spmd,
gathers the full output.

Per-core pipeline (all on-chip math fp16, PSUM accumulate f32):
  P1  cf conv (classic matmul)  -> X_off (ck-part, padded)  [inputs host-padded]
  P3  e1+off stacked conv, e2 conv -> PE-transpose -> offsets/kern px-part
  P4  softmax + deform index/weight math (px-part) -> WT int16 indices
  P2  Y_k = al_w[:,:,k] @ X_in  (swapped matmuls) -> per-tap Y^T planes in DRAM
  P5  paired-corner dma_gathers (elem=512 spans x0,x0+1; two calls y0/y1);
      bilinear MACs split BY PIXEL GROUP: g0-2 Pool STT, g3-5 DVE STT,
      g6-7 Act(mult)+DVE(add); one-tap gather prefetch pipeline
  P6  CARAFE: shifted DRAM reads of X_align^T + kern-weighted MACs (mixed)
  P7  PE-transposes move X_align / X_down into ck-part padded tiles
  P8  fo conv (classic) + LeakyReLU -> out   (chunk-pipelined under P5/P6)
"""

import numpy as np

P = 128
H = W = 64
HW = 4096
NG = 32          # px groups: px = g*128 + p
CI = 256
CO = 256
CMID = 64
K9 = 9
PADW = 66
PADHW = PADW * PADW  # 4356
NCHUNK = 8       # spatial chunks of 8 rows = 512 px for classic convs
MAGIC = 12582912.0  # 3 * 2**22, f32 round-to-int magic
# per-superchunk gather-window start rows (px rows 16s-17 .. 16s+32)
LO_S = [64 * max(0, 16 * s - 17) for s in range(4)]

# engine pipe per local pixel-group j (0..7) inside a 1024-px superchunk
#   'VP' DVE TS-mult + Pool TT-add, 'V' DVE STT chain, 'A' Act mult + DVE TT
# (Pool cannot run the fused STT op or touch PSUM on real HW)
G_PIPE = ("VP", "VP", "V", "V", "V", "A", "A", "A")
# CARAFE chain engine per local g in a 512-px chunk
CAR_PATTERN = ("V", "A", "V", "A")

_CACHE = {}
USE_LRELU = False


def _taps():
    return [(k // 3 - 1, k % 3 - 1) for k in range(9)]


def build_kernel(debug=False):
    import concourse.bass as bass
    import concourse.tile as tile
    from concourse import bacc, mybir
    from concourse.ap import AP as APc
    from concourse.masks import make_identity

    f16 = mybir.dt.float16
    f32 = mybir.dt.float32
    i16 = mybir.dt.int16
    ALU = mybir.AluOpType
    ACT = mybir.ActivationFunctionType

    nc = bacc.Bacc("TRN2", target_bir_lowering=False, debug=False, num_devices=8)

    # ---------------- DRAM I/O ----------------
    xo_d = nc.dram_tensor("xo", [P, 2 * PADHW], f16, kind="ExternalInput")
    xi_d = nc.dram_tensor("xi", [P, 2 * PADHW], f16, kind="ExternalInput")
    wcf_d = nc.dram_tensor("w_cf", [P, 9 * 4 * 2 * P], f16, kind="ExternalInput")
    wal_d = nc.dram_tensor("w_al", [P, 2 * 9 * CO], f16, kind="ExternalInput")
    weo_d = nc.dram_tensor("w_eo", [P, 9 * 2 * 82], f16, kind="ExternalInput")
    we2_d = nc.dram_tensor("w_e2", [CMID, 9 * 9], f16, kind="ExternalInput")
    wfo_d = nc.dram_tensor("w_fo", [P, 9 * 6 * 2 * P], f16, kind="ExternalInput")
    bcf_d = nc.dram_tensor("b_cf", [P, 2], f32, kind="ExternalInput")
    beo_d = nc.dram_tensor("b_eo", [82, 1], f32, kind="ExternalInput")
    be2_d = nc.dram_tensor("b_e2", [9, 1], f32, kind="ExternalInput")
    bfo_d = nc.dram_tensor("b_fo", [P, 2], f32, kind="ExternalInput")
    bcfn_d = nc.dram_tensor("b_cf_n", [P, 2], f32, kind="ExternalInput")
    bfon_d = nc.dram_tensor("b_fo_n", [P, 2], f32, kind="ExternalInput")
    alb_d = nc.dram_tensor("alb_rep", [P, CO], f16, kind="ExternalInput")
    vmask_d = nc.dram_tensor("vmask", [P, NG * 9], f16, kind="ExternalInput")
    gkiy_d = nc.dram_tensor("gkiy", [P, 9 * NG], f16, kind="ExternalInput")
    gkjx_d = nc.dram_tensor("gkjx", [P, 9 * NG], f16, kind="ExternalInput")
    out_d = nc.dram_tensor("out", [2, P, HW], f32, kind="ExternalOutput")

    taps = _taps()

    with tile.TileContext(nc) as tc:
        with (
            tc.tile_pool(name="persist", bufs=1) as pp,
            tc.tile_pool(name="dram", bufs=1, space="DRAM") as dp,
        ):
            # ---------------- persistent SBUF ----------------
            xcat = pp.tile([P, 4, PADHW], f16, tag="xcat")
            xoff = pp.tile([P, 2, PADHW], f16, tag="xoff")
            wfo = pp.tile([P, 9 * 6 * 2 * P], f16, tag="wfo")
            bcf = pp.tile([P, 2], f32, tag="bcf")
            beo = pp.tile([82, 1], f32, tag="beo")
            be2 = pp.tile([9, 1], f32, tag="be2")
            bfo = pp.tile([P, 2], f32, tag="bfo")
            bcfn = pp.tile([P, 2], f32, tag="bcfn")
            bfon = pp.tile([P, 2], f32, tag="bfon")
            albr = pp.tile([P, CO], f16, tag="albr")
            ident = pp.tile([P, P], f32, tag="ident")
            identh = pp.tile([P, P], f16, tag="identh")
            okT = pp.tile([P, NG, 27], f32, tag="okT")
            wal = pp.tile([P, 2 * 9 * CO], f16, tag="wal")
            zeros = pp.tile([P, CO], f16, tag="zeros")
            WT = pp.tile([P, 9, 256], i16, tag="WT")
            keff = pp.tile([P, NG, 9], f32, tag="keff")
            wc = [pp.tile([P, 9, NG], f32, tag=f"wc{i}", name=f"wc{i}")
                  for i in range(4)]

            # ---------------- DRAM scratch ----------------
            yt = dp.tile([9, 4226, CO], f16, tag="yt")
            xalt = dp.tile([4226, CO], f16, tag="xalt")

            def interior(padtile, ci):
                return padtile[:, ci, :].rearrange("p (h w) -> p h w", h=PADW)

            def rhs_ap(padtile, ci, r0, dy, dx, nr=8):
                # [Ppart, nr rows, 64] shifted view inside padded image
                v = interior(padtile, ci)
                return v[:, 1 + r0 + dy : 1 + r0 + nr + dy, 1 + dx : 65 + dx]

            def zero_margins(padtile, ci, npart=P):
                v = interior(padtile, ci)[0:npart]
                nc.gpsimd.memset(v[:, 0, :], 0)
                nc.gpsimd.memset(v[:, 65, :], 0)
                nc.gpsimd.memset(v[:, 1:65, 0:1], 0)
                nc.gpsimd.memset(v[:, 1:65, 65:66], 0)

            STT_V = nc.vector.scalar_tensor_tensor
            STT_P = nc.gpsimd.scalar_tensor_tensor
            TT_V = nc.vector.tensor_tensor
            TT_P = nc.gpsimd.tensor_tensor
            TS_V = nc.vector.tensor_scalar
            TS_P = nc.gpsimd.tensor_scalar

            # ---------------- P0: loads (inputs pre-padded on host) --------
            # DMA issue order matters: P1 needs xcat + wcf first.
            nc.sync.dma_start(
                xcat[:, 0:2, :].rearrange("p a b -> p (a b)"), xo_d[:])
            nc.sync.dma_start(
                xcat[:, 2:4, :].rearrange("p a b -> p (a b)"), xi_d[:])
            for ci in range(2):
                zero_margins(xoff, ci)
            make_identity(nc, ident[:])
            nc.vector.tensor_copy(identh[:], ident[:])
            nc.vector.memset(zeros[:], 0)

            def load_rest():
                nc.sync.dma_start(wfo[:], wfo_d[:])
                for sb, dr in ((bcf, bcf_d), (beo, beo_d), (be2, be2_d),
                               (bfo, bfo_d), (albr, alb_d),
                               (bcfn, bcfn_d), (bfon, bfon_d)):
                    nc.sync.dma_start(sb[:], dr[:])
                # zero margins of DRAM scratch
                for k in range(9):
                    nc.sync.dma_start(yt[k, 0:65, :], zeros[0:65, :])
                    nc.sync.dma_start(yt[k, 4161:4226, :], zeros[0:65, :])
                nc.sync.dma_start(xalt[0:65, :], zeros[0:65, 0:CO])
                nc.sync.dma_start(xalt[4161:4226, :], zeros[0:65, 0:CO])

            _ysp_cm = tc.tile_pool(name="ystage", bufs=2)
            ysp = _ysp_cm.__enter__()
            _pyy_cm = tc.tile_pool(name="py", bufs=4, space="PSUM")
            pyy = _pyy_cm.__enter__()

            # P2 band emitter: Y_k swapped matmuls -> per-tap yt planes
            ysegs = [(0, 512), (512, 512), (1024, 512), (1536, 512),
                     (2048, 256)]

            def emit_p2_band(g, late=False):
                yst = ysp.tile([P, 9, CO], f16, tag="yst", name=f"yst{g}")
                lsts = []
                for cit in range(2):
                    lst = ysp.tile([P, 128], f16, tag=f"lst{cit}",
                                   name=f"lst{cit}_{g}")
                    nc.vector.tensor_copy(
                        lst[:].rearrange("p (a b) -> p a b", a=2),
                        rhs_ap(xcat, 2 + cit, g * 2, 0, 0, nr=2),
                    )
                    lsts.append(lst)
                ystf = yst[:].rearrange("p k c -> p (k c)")
                for si, (o0, nn) in enumerate(ysegs):
                    ps = pyy.tile([P, 512], f32, tag="pyq",
                                  name=f"pyq{g}_{si}")
                    for cit in range(2):
                        nc.tensor.matmul(
                            ps[:, 0:nn],
                            lhsT=lsts[cit][:],
                            rhs=wal[:, cit * 2304 + o0 : cit * 2304 + o0 + nn],
                            start=(cit == 0),
                            stop=(cit == 1),
                        )
                    if late and si % 2 == 1:
                        nc.vector.tensor_copy(ystf[:, o0 : o0 + nn], ps[:, 0:nn])
                    else:
                        nc.scalar.activation(
                            ystf[:, o0 : o0 + nn], ps[:, 0:nn], ACT.Copy
                        )
                nc.sync.dma_start(
                    yt[:, 65 + g * 128 : 65 + (g + 1) * 128, :]
                    .transpose([1, 0, 2]),
                    yst[:],
                )

            with (
                tc.tile_pool(name="wearly", bufs=1) as wp,
                tc.tile_pool(name="stage32", bufs=2) as s32p,
                tc.tile_pool(name="pmain", bufs=2, space="PSUM") as pmm,
                tc.tile_pool(name="psmall", bufs=2, space="PSUM") as psm,
            ):
                wcf = wp.tile([P, 9 * 4 * 2 * P], f16, tag="wcf")
                weo = wp.tile([P, 9 * 2 * 82], f16, tag="weo")
                we2 = wp.tile([CMID, 9 * 9], f16, tag="we2")
                e1p = wp.tile([CMID, PADHW], f16, tag="e1p")
                nc.sync.dma_start(wcf[:], wcf_d[:])
                nc.sync.dma_start(weo[:], weo_d[:])
                nc.sync.dma_start(we2[:], we2_d[:])
                nc.sync.dma_start(wal[:], wal_d[:])
                load_rest()
                e1i = e1p[:].rearrange("p (h w) -> p h w", h=PADW)
                nc.gpsimd.memset(e1i[:, 0, :], 0)
                nc.gpsimd.memset(e1i[:, 65, :], 0)
                nc.gpsimd.memset(e1i[:, 1:65, 0:1], 0)
                nc.gpsimd.memset(e1i[:, 1:65, 65:66], 0)

                # ---------------- P1: cf conv ----------------
                for cot in range(2):
                    for c in range(NCHUNK):
                        ps = pmm.tile([P, 512], f32, tag="pmm")
                        n = 0
                        for t in range(9):
                            dy, dx = taps[t]
                            for cit in range(4):
                                j = ((t * 4 + cit) * 2 + cot) * P
                                nc.tensor.matmul(
                                    ps[:],
                                    lhsT=wcf[:, j : j + P],
                                    rhs=rhs_ap(xcat, cit, c * 8, dy, dx),
                                    start=(n == 0),
                                    stop=(n == 35),
                                )
                                n += 1
                        dstv = rhs_ap(xoff, cot, c * 8, 0, 0)
                        if USE_LRELU:
                            nc.scalar.activation(
                                dstv, ps[:].rearrange("p (a b) -> p a b", a=8),
                                ACT.Lrelu, bias=bcf[:, cot : cot + 1], alpha=0.1,
                            )
                        else:
                            nc.scalar.activation(
                                dstv, ps[:].rearrange("p (a b) -> p a b", a=8),
                                ACT.Relu, bias=bcf[:, cot : cot + 1],
                            )
                            rneg = s32p.tile([P, 512], f16, tag="st32",
                                             name=f"rncf_{cot}_{c}")
                            nc.scalar.activation(
                                rneg[:], ps[:], ACT.Relu,
                                bias=bcfn[:, cot : cot + 1], scale=-1.0,
                            )
                            STT_V(dstv, rneg[:].rearrange("p (a b) -> p a b", a=8),
                                  -0.1, dstv, ALU.mult, ALU.add)

                # ---------------- P3a: e1+off stacked conv ----------------
                for c in range(NCHUNK):
                    ps = psm.tile([82, 512], f32, tag="sm", name=f"peo_{c}")
                    n = 0
                    for t in range(9):
                        dy, dx = taps[t]
                        for cit in range(2):
                            j = (t * 2 + cit) * 82
                            nc.tensor.matmul(
                                ps[:],
                                lhsT=weo[:, j : j + 82],
                                rhs=rhs_ap(xoff, cit, c * 8, dy, dx),
                                start=(n == 0),
                                stop=(n == 17),
                            )
                            n += 1
                    nc.scalar.activation(
                        e1i[0:CMID, 1 + c * 8 : 9 + c * 8, 1:65],
                        ps[0:CMID, :].rearrange("p (a b) -> p a b", a=8),
                        ACT.Identity,
                        bias=beo[0:CMID, 0:1],
                    )
                    st = s32p.tile([18, 512], f32, tag="st32", name=f"sto_{c}")
                    nc.vector.tensor_scalar(
                        st[:], ps[CMID:82, :], beo[CMID:82, 0:1], -16.0,
                        ALU.add, ALU.max,
                    )
                    nc.vector.tensor_scalar(
                        st[:], st[:], 16.0, None, ALU.min
                    )
                    for q in range(4):
                        pt = psm.tile([P, 32], f32, tag="sm", name=f"ptr_{c}_{q}")
                        nc.tensor.transpose(
                            pt[:, 0:18], st[:, q * 128 : (q + 1) * 128],
                            ident[0:18, 0:18],
                        )
                        nc.vector.tensor_copy(okT[:, c * 4 + q, 0:18], pt[:, 0:18])

                # ---------------- P3b: e2 conv, transpose to px-part ----------
                for c in range(NCHUNK):
                    pse = psm.tile([9, 512], f32, tag="sm", name=f"pe2_{c}")
                    for t in range(9):
                        dy, dx = taps[t]
                        nc.tensor.matmul(
                            pse[:],
                            lhsT=we2[:, t * 9 : (t + 1) * 9],
                            rhs=e1i[0:CMID, 1 + c * 8 + dy : 9 + c * 8 + dy,
                                    1 + dx : 65 + dx],
                            start=(t == 0),
                            stop=(t == 8),
                        )
                    stk = s32p.tile([9, 512], f32, tag="st32", name=f"stk_{c}")
                    nc.scalar.activation(
                        stk[:], pse[:], ACT.Identity, bias=be2[:, 0:1]
                    )
                    for q in range(4):
                        pt2 = psm.tile([P, 32], f32, tag="sm", name=f"ptk_{c}_{q}")
                        nc.tensor.transpose(
                            pt2[:, 0:9], stk[:, q * 128 : (q + 1) * 128],
                            ident[0:9, 0:9],
                        )
                        nc.vector.tensor_copy(okT[:, c * 4 + q, 18:27], pt2[:, 0:9])

                # ---------------- P4: softmax + deform index math ------------
                with tc.tile_pool(name="dmath", bufs=1) as dmp:
                    vmask = dmp.tile([P, NG, 9], f16, tag="vmask")
                    gkiy = dmp.tile([P, 9, NG], f16, tag="gkiy")
                    gkjx = dmp.tile([P, 9, NG], f16, tag="gkjx")
                    nc.sync.dma_start(
                        vmask[:].rearrange("p g k -> p (g k)"), vmask_d[:])
                    nc.sync.dma_start(
                        gkiy[:].rearrange("p k g -> p (k g)"), gkiy_d[:])
                    nc.sync.dma_start(
                        gkjx[:].rearrange("p k g -> p (k g)"), gkjx_d[:])
                    expt = dmp.tile([P, NG, 9], f32, tag="expt")
                    den = dmp.tile([P, NG, 1], f32, tag="den")
                    rec = dmp.tile([P, NG, 1], f32, tag="rec")
                    ys = dmp.tile([P, 9, NG], f32, tag="ys")
                    xs = dmp.tile([P, 9, NG], f32, tag="xs")
                    y0f = dmp.tile([P, 9, NG], f32, tag="y0f")
                    x0f = dmp.tile([P, 9, NG], f32, tag="x0f")
                    fy = dmp.tile([P, 9, NG], f32, tag="fy")
                    fx = dmp.tile([P, 9, NG], f32, tag="fx")
                    tmpa = dmp.tile([P, 9, NG], f32, tag="tmpa")
                    tmpb = dmp.tile([P, 9, NG], f32, tag="tmpb")
                    wy0 = dmp.tile([P, 9, NG], f32, tag="wy0")
                    wy1 = dmp.tile([P, 9, NG], f32, tag="wy1")
                    wx0 = dmp.tile([P, 9, NG], f32, tag="wx0")
                    wx1 = dmp.tile([P, 9, NG], f32, tag="wx1")
                    idxf = dmp.tile([P, 9, NG], f32, tag="idxf")
                    tsb = [dmp.tile([P, P], f32, tag=f"tsb{j}", name=f"tsb{j}")
                           for j in range(3)]

                    nc.scalar.activation(expt[:], okT[:, :, 18:27], ACT.Exp)
                    nc.vector.tensor_reduce(den[:], expt[:],
                                            axis=mybir.AxisListType.X, op=ALU.add)
                    nc.vector.reciprocal(rec[:], den[:])
                    for g in range(NG):
                        nc.vector.tensor_scalar(
                            keff[:, g, :], expt[:, g, :], rec[:, g, 0:1],
                            None, ALU.mult
                        )
                    nc.vector.tensor_tensor(
                        out=keff[:], in0=keff[:], in1=vmask[:], op=ALU.mult
                    )

                    # offsets: okT ch 2k = dy_k, 2k+1 = dx_k ; view as [p, k, g]
                    okv = okT[:].rearrange("p g c -> p c g")
                    dys = okv[:, 0:18:2, :]
                    dxs = okv[:, 1:18:2, :]
                    fl = lambda t_: t_[:]
                    TT_V(out=fl(ys), in0=dys, in1=fl(gkiy), op=ALU.add)
                    TT_V(out=fl(xs), in0=dxs, in1=fl(gkjx), op=ALU.add)
                    TS_V(fl(y0f), fl(ys), MAGIC, MAGIC, ALU.add, ALU.subtract)
                    TS_V(fl(x0f), fl(xs), MAGIC, MAGIC, ALU.add, ALU.subtract)
                    STT_V(fl(fy), fl(ys), 0.5, fl(y0f), ALU.add, ALU.subtract)
                    STT_V(fl(fx), fl(xs), 0.5, fl(x0f), ALU.add, ALU.subtract)

                    def valid(dst, src, lo, hi):
                        TS_V(fl(tmpa), fl(src), lo, None, ALU.is_ge)
                        TS_V(fl(tmpb), fl(src), hi, None, ALU.is_le)
                        TT_V(out=fl(dst), in0=fl(tmpa), in1=fl(tmpb), op=ALU.mult)

                    valid(wy0, y0f, 0.0, 63.0)
                    valid(wy1, y0f, -1.0, 62.0)
                    valid(wx0, x0f, 0.0, 63.0)
                    valid(wx1, x0f, -1.0, 62.0)
                    TS_V(fl(tmpa), fl(fy), -1.0, 1.0, ALU.mult, ALU.add)  # 1-fy
                    TT_V(out=fl(wy0), in0=fl(wy0), in1=fl(tmpa), op=ALU.mult)
                    TT_V(out=fl(wy1), in0=fl(wy1), in1=fl(fy), op=ALU.mult)
                    TS_V(fl(tmpa), fl(fx), -1.0, 1.0, ALU.mult, ALU.add)  # 1-fx
                    TT_V(out=fl(wx0), in0=fl(wx0), in1=fl(tmpa), op=ALU.mult)
                    TT_V(out=fl(wx1), in0=fl(wx1), in1=fl(fx), op=ALU.mult)
                    TT_V(out=fl(wc[0]), in0=fl(wy0), in1=fl(wx0), op=ALU.mult)
                    TT_V(out=fl(wc[1]), in0=fl(wy0), in1=fl(wx1), op=ALU.mult)
                    TT_V(out=fl(wc[2]), in0=fl(wy1), in1=fl(wx0), op=ALU.mult)
                    TT_V(out=fl(wc[3]), in0=fl(wy1), in1=fl(wx1), op=ALU.mult)
                    # base corner row: idx = 64*clip(y0+1, 0, 64) + clip(x0+1, 0, 64)
                    TS_V(fl(tmpa), fl(y0f), 1.0, 0.0, ALU.add, ALU.max)
                    TS_V(fl(tmpa), fl(tmpa), 64.0, None, ALU.min)
                    TS_V(fl(tmpb), fl(x0f), 1.0, 0.0, ALU.add, ALU.max)
                    TS_V(fl(tmpb), fl(tmpb), 64.0, None, ALU.min)
                    STT_V(fl(idxf), fl(tmpa), 64.0, fl(tmpb), ALU.mult, ALU.add)
                    # rebase indices per superchunk gather window
                    for s in range(1, 4):
                        if LO_S[s]:
                            TS_V(idxf[:, :, s * 8 : (s + 1) * 8],
                                 idxf[:, :, s * 8 : (s + 1) * 8],
                                 float(-LO_S[s]), None, ALU.add)

                    # ---------- P4b: wrap fold idx -> [16-rep, 9, 256] int16 --
                    chunks = [(0, 4), (4, 4), (8, 1)]
                    for j, (k0, nk) in enumerate(chunks):
                        pt1 = psm.tile([P, P], f32, tag="sm", name=f"pt1_{j}")
                        nc.tensor.transpose(
                            pt1[0 : nk * 32, 0:P],
                            idxf[:, k0 : k0 + nk, :].rearrange("p a b -> p (a b)"),
                            ident[:],
                        )
                        nc.vector.tensor_copy(tsb[j][0 : nk * 32, :],
                                              pt1[0 : nk * 32, :])
                    for j, (k0, nk) in enumerate(chunks):
                        for u in range(8):
                            pt2 = psm.tile([16, P], f32, tag="sm",
                                           name=f"pt2_{j}_{u}")
                            nc.tensor.transpose(
                                pt2[:, 0 : nk * 32],
                                tsb[j][0 : nk * 32, 16 * u : 16 * u + 16],
                                ident[0 : nk * 32, 0 : nk * 32],
                            )
                            nc.vector.tensor_copy(
                                WT[0:16, k0 : k0 + nk, u : u + 249 : 8],
                                pt2[:, 0 : nk * 32]
                            )
                    for r in range(1, 8):
                        nc.sync.dma_start(
                            WT[16 * r : 16 * r + 16, :, :], WT[0:16, :, :]
                        )

                # ---- P2 (part 1): bands needed by superchunk 0 gathers ----
                for g in range(18):
                    emit_p2_band(g)

            # ======== P5..P8 pipeline over 4 superchunks of 1024 px ========
            with (
                tc.tile_pool(name="gat", bufs=3) as gdp,
                tc.tile_pool(name="macacc", bufs=2) as map_,
                tc.tile_pool(name="atmp", bufs=8) as atp,
                tc.tile_pool(name="cdst", bufs=2) as cdp,
                tc.tile_pool(name="s32b", bufs=2) as s32b,
                tc.tile_pool(name="pmain2", bufs=2, space="PSUM") as pm2,
                tc.tile_pool(name="ptr2", bufs=2, space="PSUM") as ptp,
            ):
                alck = pp.tile([P, 4, PADHW], f16, tag="xcat", name="alck")
                for ci in range(4):
                    zero_margins(alck, ci)

                def px2ck(src_slc, dsttile, plane, g, nm, eng="A"):
                    # [128 px, 128 ch] slice -> transpose -> interior rows 2g,2g+1
                    pt = ptp.tile([P, P], f16, tag="ptx", name=f"ptx_{nm}")
                    nc.tensor.transpose(pt[:], src_slc, identh[:])
                    dst = interior(dsttile, plane)[:, 1 + 2 * g : 3 + 2 * g, 1:65]
                    srcv = pt[:].rearrange("p (a b) -> p a b", a=2)
                    if eng == "A":
                        nc.scalar.activation(dst, srcv, ACT.Copy)
                    else:
                        nc.vector.tensor_copy(dst, srcv)

                def emit_carafe(c):
                    # X_down for chunk c (4 g's, 512 px) from shifted xalt reads
                    cdn = cdp.tile([P, 4, CO], f16, tag="cdn", name=f"cdn_{c}")
                    for k in range(9):
                        dy, dx = taps[k]
                        s = dy * 64 + dx
                        ct = cdp.tile([P, 4, CO], f16, tag="cd", name=f"cd_{c}_{k}")
                        nc.sync.dma_start(
                            ct[:],
                            xalt[65 + c * 512 + s : 65 + (c + 1) * 512 + s, :]
                            .rearrange("(a p) o -> p a o", p=P),
                        )
                        for gg in range(4):
                            g = c * 4 + gg
                            kw = keff[:, g, k : k + 1]
                            eng = CAR_PATTERN[gg % len(CAR_PATTERN)]
                            dst = cdn[:, gg, :]
                            src = ct[:, gg, :]
                            if eng == "VP":
                                at = atp.tile([P, CO], f16, tag="at",
                                              name=f"cvp_{c}_{k}_{gg}")
                                TS_V(at[:], src, kw, None, ALU.mult)
                                if k == 0:
                                    nc.gpsimd.tensor_copy(dst, at[:])
                                else:
                                    TT_P(out=dst, in0=dst, in1=at[:], op=ALU.add)
                            elif eng == "V":
                                if k == 0:
                                    TS_V(dst, src, kw, None, ALU.mult)
                                else:
                                    STT_V(dst, src, kw, dst, ALU.mult, ALU.add)
                            else:  # Act mult + DVE add
                                if k == 0:
                                    nc.scalar.activation(dst, src, ACT.Copy,
                                                         scale=kw)
                                else:
                                    at = atp.tile([P, CO], f16, tag="at",
                                                  name=f"cat_{c}_{k}_{gg}")
                                    nc.scalar.activation(at[:], src, ACT.Copy,
                                                         scale=kw)
                                    TT_V(out=dst, in0=dst, in1=at[:], op=ALU.add)
                    for gg in range(4):
                        for cb in range(2):
                            px2ck(cdn[:, gg, cb * 128 : (cb + 1) * 128],
                                  alck, 2 + cb, c * 4 + gg, f"d{c}_{gg}_{cb}",
                                  eng="A" if gg % 2 else "V")

                cat2 = [(xoff, 0), (xoff, 1), (alck, 0), (alck, 1),
                        (alck, 2), (alck, 3)]

                def emit_fo_chunk(c):
                    for cot in range(2):
                        ps = pm2.tile([P, 512], f32, tag="pmm2",
                                      name=f"pfo_{c}_{cot}")
                        n = 0
                        for t in range(9):
                            dy, dx = taps[t]
                            for cit in range(6):
                                src, ci = cat2[cit]
                                j = ((t * 6 + cit) * 2 + cot) * P
                                nc.tensor.matmul(
                                    ps[:],
                                    lhsT=wfo[:, j : j + P],
                                    rhs=rhs_ap(src, ci, c * 8, dy, dx),
                                    start=(n == 0),
                                    stop=(n == 53),
                                )
                                n += 1
                        ost = s32b.tile([P, 512], f32, tag="ost",
                                        name=f"ost_{cot}_{c}")
                        if USE_LRELU:
                            nc.scalar.activation(
                                ost[:], ps[:], ACT.Lrelu,
                                bias=bfo[:, cot : cot + 1], alpha=0.1,
                            )
                        else:
                            nc.scalar.activation(
                                ost[:], ps[:], ACT.Relu, bias=bfo[:, cot : cot + 1],
                            )
                            rneg2 = s32b.tile([P, 512], f32, tag="ost",
                                              name=f"rnfo_{cot}_{c}")
                            nc.scalar.activation(
                                rneg2[:], ps[:], ACT.Relu,
                                bias=bfon[:, cot : cot + 1], scale=-1.0,
                            )
                            STT_V(ost[:], rneg2[:], -0.1, ost[:], ALU.mult, ALU.add)
                        nc.sync.dma_start(
                            out_d[cot, :, c * 512 : (c + 1) * 512], ost[:]
                        )

                gsem = nc.alloc_semaphore("gat_sem")
                # superchunk gather windows (indices rebased by LO_S in P4):
                # W = max rebased idx + 1; window rows [lo+d, lo+d+W+1)
                IDX_MAX = [64 * min(64, 16 * s + 33) + 64 for s in range(4)]
                W_S = [IDX_MAX[s] - LO_S[s] + 1 for s in range(4)]

                def emit_gathers(s, k):
                    # Two preps per (s, k): y-corner pairs via d-offset base;
                    # in_ap windowed to rows [lo_s+d, lo_s+d+W_s) so the tile
                    # dep only covers the yt bands this superchunk reads.
                    gt = gdp.tile([P, 16, 512], f16, tag="gd",
                                  name=f"gd_{s}_{k}")
                    for yc, d in ((0, 0), (1, 64)):
                        v = yt[k, LO_S[s] + d : LO_S[s] + d + W_S[s], :]
                        v2 = APc(v.tensor, v.offset, [[256, W_S[s]], [1, 512]])
                        nc.gpsimd.dma_gather(
                            gt[:, 8 * yc : 8 * (yc + 1), :], v2,
                            WT[:, k, s * 64 : (s + 1) * 64],
                            1024, 1024, 512, elem_step=256,
                        )
                    return gt

                def emit_macs(s, k, gt, xal_s):
                    for j in range(8):
                        g = s * 8 + j
                        vv = [gt[:, j, 0:256], gt[:, j, 256:512],
                              gt[:, 8 + j, 0:256], gt[:, 8 + j, 256:512]]
                        pipe = G_PIPE[j]
                        dst = xal_s[:, j, :]
                        if pipe == "VP":
                            vt = atp.tile([P, 4, CO], f16, tag="vt",
                                          name=f"vp_{s}_{k}_{j}")
                            for cor in range(4):
                                TS_V(vt[:, cor, :], vv[cor],
                                     wc[cor][:, k, g : g + 1], None, ALU.mult)
                            for cor in range(4):
                                if k == 0 and cor == 0:
                                    TT_P(out=dst, in0=vt[:, 0, :], in1=albr[:],
                                         op=ALU.add)
                                else:
                                    TT_P(out=dst, in0=dst, in1=vt[:, cor, :],
                                         op=ALU.add)
                            continue
                        for cor in range(4):
                            w_ = wc[cor][:, k, g : g + 1]
                            first = k == 0 and cor == 0
                            if pipe == "V":
                                if first:
                                    STT_V(dst, vv[cor], w_, albr[:],
                                          ALU.mult, ALU.add)
                                else:
                                    STT_V(dst, vv[cor], w_, dst,
                                          ALU.mult, ALU.add)
                            elif pipe == "VP":
                                pass  # handled batched below
                            else:
                                at = atp.tile([P, CO], f16, tag="at",
                                              name=f"at_{s}_{k}_{j}_{cor}")
                                nc.scalar.activation(at[:], vv[cor],
                                                     ACT.Copy, scale=w_)
                                if first:
                                    TT_V(out=dst, in0=at[:], in1=albr[:],
                                         op=ALU.add)
                                else:
                                    TT_V(out=dst, in0=dst, in1=at[:],
                                         op=ALU.add)

                P2_BANDS = {1: range(18, 26), 2: range(26, 32)}
                for s in range(4):
                    for g in P2_BANDS.get(s, ()):
                        emit_p2_band(g, late=True)
                    xal_s = map_.tile([P, 8, CO], f16, tag="xal", name=f"xal{s}")
                    gts = [emit_gathers(s, 0), emit_gathers(s, 1)]
                    for k in range(9):
                        if k < 7:
                            gts.append(emit_gathers(s, k + 2))
                        emit_macs(s, k, gts[k], xal_s)
                        gts[k] = None
                    # store 2 bands of 512 px to xalt (for CARAFE shifts)
                    for h in range(2):
                        c = 2 * s + h
                        nc.sync.dma_start(
                            xalt[65 + c * 512 : 65 + (c + 1) * 512, :]
                            .rearrange("(a p) o -> p a o", p=P),
                            xal_s[:, h * 4 : (h + 1) * 4, :],
                        )
                    # PE-transpose X_align into ck-part padded tile
                    for j in range(8):
                        for cb in range(2):
                            px2ck(xal_s[:, j, cb * 128 : (cb + 1) * 128],
                                  alck, cb, s * 8 + j, f"a{s}_{j}_{cb}",
                                  eng="A" if j % 2 else "V")
                    # trailing CARAFE / fo conv
                    if s == 0:
                        emit_carafe(0)
                    else:
                        emit_carafe(2 * s - 1)
                        emit_carafe(2 * s)
                        emit_fo_chunk(2 * s - 2)
                        emit_fo_chunk(2 * s - 1)
                # tail
                emit_carafe(7)
                emit_fo_chunk(6)
                emit_fo_chunk(7)
            _ysp_cm.__exit__(None, None, None)
            _pyy_cm.__exit__(None, None, None)

    nc.compile()
    return nc


def pack_inputs(inputs):
    """Host-side prep: per-core in_maps from full inputs."""
    f = np.float16
    X_O = np.asarray(inputs["X_O"], np.float32)
    X_in = np.asarray(inputs["X_in"], np.float32)
    B = X_O.shape[0]

    def conv_w(w, s=None):
        w = np.asarray(w, np.float32)
        if s is not None:
            w = w * np.asarray(s, np.float32)[:, None, None, None]
        return w

    cf_w = conv_w(inputs["cf_w"], inputs["cf_s"])
    off_w = conv_w(inputs["off_w"])
    al_w = conv_w(inputs["al_w"])
    e1_w = conv_w(inputs["e1_w"], inputs["e1_s"])
    e2_w = conv_w(inputs["e2_w"], inputs["e2_s"])
    fo_w = conv_w(inputs["fo_w"], inputs["fo_s"])

    # w_cf: [p, t, cit, cot, co] ; w[o, c, ky, kx], c = cit*128+p, o = cot*128+co
    w = cf_w.reshape(2, P, 4, P, 9)  # [cot, co, cit, p, t]
    w_cf = np.ascontiguousarray(w.transpose(3, 4, 2, 0, 1)).reshape(P, -1).astype(f)
    w = al_w.reshape(CO, 2, P, 9)  # [o, cit, p, t]
    w_al = np.ascontiguousarray(w.transpose(2, 1, 3, 0)).reshape(P, -1).astype(f)
    # stacked e1 (64 out) + off (18 out): [p, t, cit, 82]
    w_e1 = e1_w.reshape(CMID, 2, P, 9).transpose(2, 3, 1, 0)  # [p, t, cit, 64]
    w_of = off_w.reshape(18, 2, P, 9).transpose(2, 3, 1, 0)   # [p, t, cit, 18]
    w_eo = np.concatenate([w_e1, w_of], axis=3)
    w_eo = np.ascontiguousarray(w_eo).reshape(P, -1).astype(f)
    w = e2_w.reshape(9, CMID, 9)
    w_e2 = np.ascontiguousarray(w.transpose(1, 2, 0)).reshape(CMID, -1).astype(f)
    w = fo_w.reshape(2, P, 6, P, 9)
    w_fo = np.ascontiguousarray(w.transpose(3, 4, 2, 0, 1)).reshape(P, -1).astype(f)

    b_cf = np.asarray(inputs["cf_sh"], np.float32).reshape(2, P).T.copy()
    b_eo = np.concatenate([
        np.asarray(inputs["e1_sh"], np.float32).reshape(CMID),
        np.asarray(inputs["off_b"], np.float32).reshape(18),
    ]).reshape(82, 1)
    b_e2 = np.asarray(inputs["e2_sh"], np.float32).reshape(9, 1)
    b_fo = np.asarray(inputs["fo_sh"], np.float32).reshape(2, P).T.copy()
    b_cf_n = -b_cf
    b_fo_n = -b_fo
    alb_rep = np.broadcast_to(
        np.asarray(inputs["al_b"], np.float32), (P, CO)
    ).astype(f).copy()

    # constants: px = g*128 + p ; y = px//64, x = px%64
    pxs = np.arange(HW)
    ppx = pxs.reshape(NG, P)  # [g, p]
    yy = (ppx // 64).T  # [p, g]
    xx = (ppx % 64).T
    ki = np.array([t // 3 - 1 for t in range(9)])
    kj = np.array([t % 3 - 1 for t in range(9)])
    gkiy = (yy[:, None, :] + ki[None, :, None] - 0.5).astype(np.float16).reshape(P, -1)
    gkjx = (xx[:, None, :] + kj[None, :, None] - 0.5).astype(np.float16).reshape(P, -1)
    vy = (yy[:, None, :] + ki[None, :, None] >= 0) & (
        yy[:, None, :] + ki[None, :, None] <= 63
    )
    vx = (xx[:, None, :] + kj[None, :, None] >= 0) & (
        xx[:, None, :] + kj[None, :, None] <= 63
    )
    vmask = (vy & vx).transpose(0, 2, 1).astype(f).reshape(P, -1)  # [p, g, k]

    def pad_planes(x):
        # [2, 128, 4096] -> host-padded [128, 2, 66, 66] -> [128, 2*4356]
        out = np.zeros((2, P, PADW, PADW), np.float32)
        out[:, :, 1:65, 1:65] = x.reshape(2, P, 64, 64)
        return np.ascontiguousarray(
            out.transpose(1, 0, 2, 3).reshape(P, -1)).astype(f)

    shared = dict(
        w_cf=w_cf, w_al=w_al, w_eo=w_eo, w_e2=w_e2, w_fo=w_fo,
        b_cf=b_cf, b_eo=b_eo, b_e2=b_e2, b_fo=b_fo,
        b_cf_n=b_cf_n, b_fo_n=b_fo_n,
        alb_rep=alb_rep, vmask=vmask, gkiy=gkiy, gkjx=gkjx,
    )
    in_maps = []
    for b in range(B):
        m = dict(shared)
        m["xo"] = pad_planes(X_O[b].reshape(2, P, HW))
        m["xi"] = pad_planes(X_in[b].reshape(2, P, HW))
        in_maps.append(m)
    return in_maps


def kernel(**inputs):
    from concourse.bass_utils import run_bass_kernel_spmd

    if "nc" not in _CACHE:
        _CACHE["nc"] = build_kernel()
    nc = _CACHE["nc"]
    in_maps = pack_inputs(inputs)
    B = len(in_maps)
    res = run_bass_kernel_spmd(nc, in_maps, core_ids=list(range(B)))
    outs = [
        res.results[b]["out"].reshape(CO, H, W).astype(np.float32) for b in range(B)
    ]
    return np.stack(outs, axis=0)
